# revision 1
# baseline (speedup 1.0000x reference)
"""Bass/Trainium2 kernel for nn_AttentionMemory (scatter_memory).

Reference computation (per batch b):
    S   = Mk^T @ Qk * (1/sqrt(CK))     # [HW, HW]
    P   = softmax(S, axis=memory)      # softmax over the m (row) axis
    out = mv @ P                       # [CV, HW]

Sharding: B=8 batches, one batch per NeuronCore (pure data parallel).

Per-core algorithm (HW=4096, CK=64, CV=512):
  - mk/qk cast to fp16 once (DVE), duplicated into both partition halves so
    S-matmuls (K=64) run pairwise-concurrent in the two PE row-halves.
  - mvT = mv^T via PE transposes (bf16), once.
  - For each q-group of 512 columns:
      * S[m,q] chunk tiles via fp16 matmuls (lhsT=mk chunk, rhs=qk group)
      * P = exp(S * scale) on ScalarE straight out of PSUM (bf16; bf16 is
        needed for range: S spans ~±205 here so exp reaches ~1e11).
        No max subtraction: softmax is shift-invariant and fp32/bf16 hold
        the range, so the result matches the reference.
      * Z[q] = colsum(P) via ones-vector matmuls, 4 chains packed into
        distinct PE column-groups (M=1 each) to run concurrently.
      * out_raw[c,q] = mvT^T @ P (bf16 matmul, fp32 accumulate)
      * out = out_raw * (1/Z) broadcast along partitions, DMA out.
"""

import numpy as np

import concourse.bass as bass
import concourse.mybir as mybir
import concourse.tile as tile
from concourse.masks import make_identity
from bass_rust import ScopedClock

B, CK, CV, H, W = 8, 64, 512, 64, 64
HW = H * W            # 4096
QG = 512              # q-group width (one PSUM bank of fp32)
NQ = HW // QG         # 8 q-groups
NM = HW // 128        # 32 m-chunks
NCB = CV // 128       # 4 c-blocks
SCALE = 1.0 / 8.0     # 1/sqrt(CK)

F32 = mybir.dt.float32
FP16 = mybir.dt.float16
BF16 = mybir.dt.bfloat16

PACK_S = True   # run S-matmul pairs in the two K=64 row-halves of the PE
PACK_Z = True   # run 4 Z-accumulation chains in distinct PE column-groups


class FixedTileContext(tile.TileContext):
    """Splits multi-wait sync_infos: this walrus accepts at most one sync
    wait per regular instruction (two on InstEventSemaphore). Extra waits
    move onto same-engine InstNoOp carriers inserted just before."""

    def _split_multi_waits(self, ordered):
        nc = self.nc
        for bb_name, insts in list(ordered.items()):
            new_insts = []
            changed = False
            for inst in insts:
                si = getattr(inst, "sync_info", None)
                waits = list(si.on_wait) if (si is not None and si.on_wait) else []
                limit = 2 if isinstance(inst, mybir.InstEventSemaphore) else 1
                if len(waits) > limit:
                    changed = True
                    for w in waits[limit:]:
                        new_insts.append(
                            mybir.InstNoOp(
                                name=nc.get_next_instruction_name(),
                                sync_info=mybir.SyncInfo(on_wait=[w], on_update=[]),
                                bass_nofuse=True,
                                engine=inst.engine,
                            )
                        )
                    inst.sync_info = mybir.SyncInfo(
                        on_wait=waits[:limit], on_update=list(si.on_update or [])
                    )
                new_insts.append(inst)
            if changed:
                ordered[bb_name] = new_insts

    def _lower_ordered_insts(self, ordered):
        self._split_multi_waits(ordered)
        return super()._lower_ordered_insts(ordered)

    def _drain_and_barrier(self, tick_clock, wait_clock):
        nc = self.nc
        drain_inst = nc.sync.drain()
        wait_clock.add_sem_waits(
            drain_inst.ins, ScopedClock({None: tick_clock.global_clock})
        )
        si = drain_inst.ins.sync_info
        waits = list(si.on_wait or []) if si is not None else []
        if len(waits) > 1:
            drain_inst.ins.sync_info = mybir.SyncInfo(
                on_wait=[waits[0]], on_update=list(si.on_update or [])
            )
            for w in waits[1:]:
                d2 = nc.sync.drain()
                d2.ins.sync_info = mybir.SyncInfo(on_wait=[w], on_update=[])
        nc.all_engine_barrier()
        assert self.sems is not None
        popped = nc._tile_sem_poison_stack.pop()
        assert popped is self._sem_poison
        nc.clear_and_free_semaphores(list(self.sems.allocated().values()))
        nc.all_engine_barrier()


def build_program(repeat: int = 1) -> bass.Bass:
    nc = bass.Bass()
    mk_d = nc.dram_tensor("Mk", [CK, HW], F32, kind="ExternalInput")
    qk_d = nc.dram_tensor("Qk", [CK, HW], F32, kind="ExternalInput")
    mv_d = nc.dram_tensor("mv", [CV, HW], F32, kind="ExternalInput")
    out_d = nc.dram_tensor("out", [CV, HW], F32, kind="ExternalOutput")

    with FixedTileContext(nc) as tc:
        with (
            tc.tile_pool(name="consts", bufs=1) as consts,
            tc.tile_pool(name="stage", bufs=2) as stage,
            tc.tile_pool(name="inp16", bufs=1) as inp16,
            tc.tile_pool(name="mvtp", bufs=1) as mvtp,
            tc.tile_pool(name="pp", bufs=2) as pp,
            tc.tile_pool(name="obp", bufs=3) as obp,
            tc.tile_pool(name="smallp", bufs=2) as smallp,
            tc.tile_pool(name="ps_s", bufs=2, space="PSUM") as ps_s_pool,
            tc.tile_pool(name="ps_o", bufs=4, space="PSUM") as ps_o_pool,
            tc.tile_pool(name="ps_z", bufs=1, space="PSUM") as ps_z_pool,
            tc.tile_pool(name="ps_r", bufs=1, space="PSUM") as ps_r_pool,
        ):
            identity = consts.tile([128, 128], F32)
            make_identity(nc, identity[:])
            ones_h = consts.tile([128, 1], BF16)
            nc.gpsimd.memset(ones_h[:], 1.0)
            ones_r = consts.tile([1, 128], F32)
            nc.gpsimd.memset(ones_r[:], 1.0)

            for _rep in range(repeat):
                # Load + cast mk, qk to fp16 (duplicated across partition halves
                # when PACK_S so the two PE row-halves can run concurrently).
                # repeat>1 re-runs the whole pipeline (for overhead-free timing).
                KP = 128 if PACK_S else CK
                mk16 = inp16.tile([KP, HW], FP16)
                qk16 = inp16.tile([KP, HW], FP16)
                st_mk = stage.tile([128, HW], F32, tag="stage")
                nc.sync.dma_start(st_mk[:CK, :], mk_d[:])
                nc.vector.tensor_copy(mk16[:CK, :], st_mk[:CK, :])
                st_qk = stage.tile([128, HW], F32, tag="stage")
                nc.sync.dma_start(st_qk[:CK, :], qk_d[:])
                nc.vector.tensor_copy(qk16[:CK, :], st_qk[:CK, :])
                if PACK_S:
                    nc.vector.tensor_copy(mk16[CK:, :], st_mk[:CK, :])
                    nc.vector.tensor_copy(qk16[CK:, :], st_qk[:CK, :])

                def s_phase(g):
                    """S matmuls + exp + Z accumulation for q-group g."""
                    qsl = slice(g * QG, (g + 1) * QG)
                    P = pp.tile([128, NM, QG], BF16, tag="P")
                    zw = 128 if PACK_Z else 1
                    ps_z = ps_z_pool.tile([zw, QG], F32, tag="z")
                    for j in range(NM):
                        half = (j % 2) if PACK_S else 0
                        ksl = slice(half * CK, half * CK + CK)
                        ps_sj = ps_s_pool.tile([128, QG], F32, tag="s")
                        nc.tensor.matmul(
                            ps_sj[:],
                            mk16[ksl, j * 128:(j + 1) * 128],
                            qk16[ksl, qsl],
                            start=True,
                            stop=True,
                        )
                        nc.scalar.activation(
                            P[:, j, :], ps_sj[:],
                            mybir.ActivationFunctionType.Exp, scale=SCALE,
                        )
                        # Z[q] += colsum of this chunk (keeps PE busy during exps)
                        if PACK_Z:
                            col = j % 4
                            nc.tensor.matmul(
                                ps_z[32 * col:32 * col + 1, :], ones_h[:],
                                P[:, j, :],
                                start=(j < 4), stop=(j >= NM - 4),
                                tile_position=(0, 32 * col),
                            )
                        else:
                            nc.tensor.matmul(
                                ps_z[:], ones_h[:], P[:, j, :],
                                start=(j == 0), stop=(j == NM - 1),
                            )
                    return qsl, P, ps_z

                # mvT[p, j, c] = mv[c, j*128+p], bf16
                mvT = mvtp.tile([128, NM, CV], BF16)
                for cb in range(NCB):
                    mv_sb = stage.tile([128, HW], F32, tag="stage")
                    nc.sync.dma_start(mv_sb[:], mv_d[cb * 128:(cb + 1) * 128, :])
                    for j in range(NM):
                        ps_t = ps_o_pool.tile([128, 128], F32, tag="o")
                        nc.tensor.transpose(
                            ps_t[:], mv_sb[:, j * 128:(j + 1) * 128], identity[:]
                        )
                        nc.vector.tensor_copy(
                            mvT[:, j, cb * 128:(cb + 1) * 128], ps_t[:]
                        )

                for g in range(NQ):
                    qsl, P, ps_z = s_phase(g)

                    rz = smallp.tile([1, QG], F32, tag="rz")
                    if PACK_Z:
                        # combine the 4 column-group partials; VectorE may read
                        # at most one PSUM operand per instruction, so chain
                        # through SBUF.
                        za = smallp.tile([1, QG], F32, tag="za")
                        nc.vector.tensor_copy(za[:], ps_z[0:1, :])
                        zb = smallp.tile([1, QG], F32, tag="zb")
                        nc.vector.tensor_tensor(
                            out=zb[:], in0=za[:], in1=ps_z[32:33, :],
                            op=mybir.AluOpType.add,
                        )
                        zc = smallp.tile([1, QG], F32, tag="zc")
                        nc.vector.tensor_tensor(
                            out=zc[:], in0=zb[:], in1=ps_z[64:65, :],
                            op=mybir.AluOpType.add,
                        )
                        zs = smallp.tile([1, QG], F32, tag="zs")
                        nc.vector.tensor_tensor(
                            out=zs[:], in0=zc[:], in1=ps_z[96:97, :],
                            op=mybir.AluOpType.add,
                        )
                        nc.vector.reciprocal(rz[:], zs[:])
                    else:
                        nc.vector.reciprocal(rz[:], ps_z[:])

                    # broadcast rz along partitions: ones[1,128]^T @ rz[1,QG]
                    ps_rzb = ps_r_pool.tile([128, QG], F32, tag="rzb")
                    nc.tensor.matmul(
                        ps_rzb[:], ones_r[:], rz[:], start=True, stop=True,
                    )
                    rzb = smallp.tile([128, QG], F32, tag="rzb_sb")
                    nc.vector.tensor_copy(rzb[:], ps_rzb[:])

                    for cb in range(NCB):
                        ps_o = ps_o_pool.tile([128, QG], F32, tag="o")
                        for j in range(NM):
                            nc.tensor.matmul(
                                ps_o[:],
                                mvT[:, j, cb * 128:(cb + 1) * 128],
                                P[:, j, :],
                                start=(j == 0),
                                stop=(j == NM - 1),
                            )
                        o_sb = obp.tile([128, QG], F32, tag="ob")
                        nc.vector.tensor_tensor(
                            out=o_sb[:], in0=ps_o[:], in1=rzb[:],
                            op=mybir.AluOpType.mult,
                        )
                        nc.sync.dma_start(
                            out_d[cb * 128:(cb + 1) * 128, qsl], o_sb[:]
                        )
    return nc


_prog_cache = None


def _get_program():
    global _prog_cache
    if _prog_cache is None:
        _prog_cache = build_program()
    return _prog_cache


def run(inputs, **spmd_kwargs):
    from concourse.bass_utils import run_bass_kernel_spmd

    Mk = np.ascontiguousarray(np.asarray(inputs["Mk"], dtype=np.float32))
    Qk = np.ascontiguousarray(np.asarray(inputs["Qk"], dtype=np.float32))
    mv = np.ascontiguousarray(np.asarray(inputs["mv"], dtype=np.float32))
    assert Mk.shape == (B, CK, H, W) and Qk.shape == (B, CK, H, W)
    assert mv.shape == (B, CV, H, W)

    in_maps = [
        {
            "Mk": Mk[b].reshape(CK, HW),
            "Qk": Qk[b].reshape(CK, HW),
            "mv": mv[b].reshape(CV, HW),
        }
        for b in range(B)
    ]
    nc = _get_program()
    res = run_bass_kernel_spmd(nc, in_maps, list(range(B)), **spmd_kwargs)
    out = np.stack([res.results[b]["out"] for b in range(B)])
    return out.reshape(B, CV, H, W).astype(np.float32), res


def kernel(**inputs) -> np.ndarray:
    out, _ = run(inputs)
    return out



# revision 3
# speedup vs baseline: 243.7888x; 243.7888x over previous
"""Bass/Trainium2 kernel for nn_AttentionMemory (scatter_memory), v2.

Reference computation (per batch b):
    S   = Mk^T @ Qk * (1/sqrt(CK))     # [HW, HW]
    P   = softmax(S, axis=memory)      # softmax over the m (row) axis
    out = mv @ P                       # [CV, HW]

Sharding: B=8 batches, one batch per NeuronCore (pure data parallel).

v2 schedule: fine-grained slot interleave. The S/exp/Z stream for group
g+1 is woven between the PV accumulation matmuls of group g so the
Activation engine (exp, 612 ns/tile) runs concurrently with PE instead of
gating a separate S phase. PV chains are staggered across "flat slots"
(chain (g,cb) occupies flat slots 32g+8cb .. +15 at 2 matmuls/slot) so
PSUM drains + out-multiplies spread out instead of bunching at group
boundaries. Z colsums ride in distinct PE column groups (tile_position)
and S matmuls in the two K=64 row halves, which run concurrently on HW.
rz broadcast moved to the idle GPSIMD engine (partition_broadcast).
"""

import numpy as np

import concourse.bass as bass
import concourse.mybir as mybir
import concourse.tile as tile
from concourse.masks import make_identity
from bass_rust import ScopedClock

B, CK, CV, H, W = 8, 64, 512, 64, 64
HW = H * W            # 4096
QG = 512              # q-group width (one PSUM bank of fp32)
NQ = HW // QG         # 8 q-groups
NM = HW // 128        # 32 m-chunks
NCB = CV // 128       # 4 c-blocks
SCALE = 1.0 / 8.0     # 1/sqrt(CK)

F32 = mybir.dt.float32
FP16 = mybir.dt.float16
BF16 = mybir.dt.bfloat16


class FixedTileContext(tile.TileContext):
    """Splits multi-wait sync_infos: this walrus accepts at most one sync
    wait per regular instruction (two on InstEventSemaphore). Extra waits
    move onto same-engine InstNoOp carriers inserted just before."""

    def _split_multi_waits(self, ordered):
        nc = self.nc
        for bb_name, insts in list(ordered.items()):
            new_insts = []
            changed = False
            for inst in insts:
                si = getattr(inst, "sync_info", None)
                waits = list(si.on_wait) if (si is not None and si.on_wait) else []
                limit = 2 if isinstance(inst, mybir.InstEventSemaphore) else 1
                if len(waits) > limit:
                    changed = True
                    for w in waits[limit:]:
                        new_insts.append(
                            mybir.InstNoOp(
                                name=nc.get_next_instruction_name(),
                                sync_info=mybir.SyncInfo(on_wait=[w], on_update=[]),
                                bass_nofuse=True,
                                engine=inst.engine,
                            )
                        )
                    inst.sync_info = mybir.SyncInfo(
                        on_wait=waits[:limit], on_update=list(si.on_update or [])
                    )
                new_insts.append(inst)
            if changed:
                ordered[bb_name] = new_insts

    def _lower_ordered_insts(self, ordered):
        self._split_multi_waits(ordered)
        return super()._lower_ordered_insts(ordered)

    def _drain_and_barrier(self, tick_clock, wait_clock):
        nc = self.nc
        drain_inst = nc.sync.drain()
        wait_clock.add_sem_waits(
            drain_inst.ins, ScopedClock({None: tick_clock.global_clock})
        )
        si = drain_inst.ins.sync_info
        waits = list(si.on_wait or []) if si is not None else []
        if len(waits) > 1:
            drain_inst.ins.sync_info = mybir.SyncInfo(
                on_wait=[waits[0]], on_update=list(si.on_update or [])
            )
            for w in waits[1:]:
                d2 = nc.sync.drain()
                d2.ins.sync_info = mybir.SyncInfo(on_wait=[w], on_update=[])
        nc.all_engine_barrier()
        assert self.sems is not None
        popped = nc._tile_sem_poison_stack.pop()
        assert popped is self._sem_poison
        nc.clear_and_free_semaphores(list(self.sems.allocated().values()))
        nc.all_engine_barrier()


def build_program(repeat: int = 1) -> bass.Bass:
    nc = bass.Bass()
    mk_d = nc.dram_tensor("Mk", [CK, HW], F32, kind="ExternalInput")
    qk_d = nc.dram_tensor("Qk", [CK, HW], F32, kind="ExternalInput")
    mv_d = nc.dram_tensor("mv", [CV, HW], F32, kind="ExternalInput")
    out_d = nc.dram_tensor("out", [CV, HW], F32, kind="ExternalOutput")

    with FixedTileContext(nc) as tc:
        with (
            tc.tile_pool(name="consts", bufs=1) as consts,
            tc.tile_pool(name="stage", bufs=2) as stage,
            tc.tile_pool(name="inp16", bufs=1) as inp16,
            tc.tile_pool(name="mvtp", bufs=1) as mvtp,
            tc.tile_pool(name="pp", bufs=3) as pp,
            tc.tile_pool(name="obp", bufs=2) as obp,
            tc.tile_pool(name="smallp", bufs=2) as smallp,
            tc.tile_pool(name="ps", bufs=2, space="PSUM") as ps,
        ):
            identity = consts.tile([128, 128], F32)
            make_identity(nc, identity[:])

            ones_h = consts.tile([128, 1], BF16)
            nc.gpsimd.memset(ones_h[:], 1.0)
            ones_r = consts.tile([1, 128], BF16)
            nc.gpsimd.memset(ones_r[:], 1.0)

            for _rep in range(repeat):
                emit_body(nc, tc, stage, inp16, mvtp, pp, obp, smallp, ps,
                          identity, ones_h, ones_r, mk_d, qk_d, mv_d, out_d)
    return nc


def emit_body(nc, tc, stage, inp16, mvtp, pp, obp, smallp, ps,
              identity, ones_h, ones_r, mk_d, qk_d, mv_d, out_d):
    # ---- input load + cast (fp16, duplicated into both partition halves
    # via double-DMA so the two K=64 PE row-halves run S-pairs concurrently)
    mk16 = inp16.tile([128, HW], FP16)
    qk16 = inp16.tile([128, HW], FP16)
    NCH = 4
    CW = HW // NCH
    mv_sb = []
    for ch in range(NCH):
        csl = slice(ch * CW, (ch + 1) * CW)
        for src, dst in ((mk_d, mk16), (qk_d, qk16)):
            st = stage.tile([128, CW], F32, tag="mkqk")
            nc.sync.dma_start(st[:CK, :], src[:, csl])
            nc.sync.dma_start(st[CK:, :], src[:, csl])
            nc.vector.tensor_copy(dst[:, csl], st[:])
    # mv staging: same SP queue, emitted after the chunk DMAs so the
    # latency-critical mk/qk chunk transfers always run first
    for cb in range(NCB):
        t = stage.tile([128, HW], F32, tag="mv", name=f"mv_sb{cb}")
        nc.sync.dma_start(t[:], mv_d[cb * 128:(cb + 1) * 128, :])
        mv_sb.append(t)

    # mvT[p, j, c] = mv[c, j*128+p], bf16 (PV stationary operand)
    mvT = mvtp.tile([128, NM, CV], BF16)

    P = [None] * NQ     # P[g]: [128, NM, QG] bf16, unnormalized exp
    ps_z = [None] * NQ  # Z colsum accumulators (4 col-group partials)
    rzb = [None] * NQ   # broadcast 1/Z rows
    ps_o = {}           # (g, cb) -> PV accumulation PSUM tile

    def emit_transpose_quad(cb, q):
        """Transpose m-chunks j=4q..4q+3 of mv c-block cb: 4 PE transposes
        into one PSUM tile, one DVE copy out (keeps the shared s-ring at
        ~2 allocs/slot)."""
        ps_t = ps.tile([128, QG], F32, tag="s", name="ps_t")
        for jj in range(4):
            j4 = 4 * q + jj
            nc.tensor.transpose(
                ps_t[:, jj * 128:(jj + 1) * 128],
                mv_sb[cb][:, j4 * 128:(j4 + 1) * 128], identity[:]
            )
        nc.vector.tensor_copy(
            mvT[:, 4 * q:4 * q + 4, cb * 128:(cb + 1) * 128],
            ps_t.rearrange("p (j c) -> p j c", j=4),
        )

    def emit_s(g, j):
        """One S matmul + exp for (g, j). Allocates P[g]/ps_z[g] on j==0."""
        if j == 0:
            P[g] = pp.tile([128, NM, QG], BF16, tag="P", name=f"P{g}")
            ps_z[g] = ps.tile([128, QG], F32, tag="z", name=f"ps_z{g}")
        qsl = slice(g * QG, (g + 1) * QG)
        half = j % 2
        ksl = slice(half * CK, half * CK + CK)
        ps_sj = ps.tile([128, QG], F32, tag="s", name="ps_s")
        nc.tensor.matmul(
            ps_sj[:], mk16[ksl, j * 128:(j + 1) * 128], qk16[ksl, qsl],
            start=True, stop=True,
        )
        nc.scalar.activation(
            P[g][:, j, :], ps_sj[:],
            mybir.ActivationFunctionType.Exp, scale=SCALE,
        )

    def emit_z_quad(g, a):
        """Z colsum chunks j=4a..4a+3 for group g, emitted back-to-back so
        the 4 column-group chains run concurrently on the PE array."""
        for c in range(4):
            nc.tensor.matmul(
                ps_z[g][32 * c:32 * c + 1, :], ones_h[:], P[g][:, 4 * a + c, :],
                start=(a == 0), stop=(a == NM // 4 - 1),
                tile_position=(0, 32 * c),
            )

    def emit_rz(g):
        """Combine Z partials -> reciprocal -> broadcast (DVE + GPSIMD)."""
        za = smallp.tile([1, QG], F32, tag="zt", name="za")
        nc.vector.tensor_copy(za[:], ps_z[g][0:1, :])
        zb = smallp.tile([1, QG], F32, tag="zt", name="zb")
        nc.vector.tensor_tensor(
            out=zb[:], in0=za[:], in1=ps_z[g][32:33, :], op=mybir.AluOpType.add
        )
        zc = smallp.tile([1, QG], F32, tag="zt", name="zc")
        nc.vector.tensor_tensor(
            out=zc[:], in0=zb[:], in1=ps_z[g][64:65, :], op=mybir.AluOpType.add
        )
        zs = smallp.tile([1, QG], F32, tag="zt", name="zs")
        nc.vector.tensor_tensor(
            out=zs[:], in0=zc[:], in1=ps_z[g][96:97, :], op=mybir.AluOpType.add
        )
        rz = smallp.tile([1, QG], F32, tag="rz", name="rz")
        nc.vector.reciprocal(rz[:], zs[:])
        rz16 = smallp.tile([1, QG], BF16, tag="rz16", name="rz16")
        nc.vector.tensor_copy(rz16[:], rz[:])
        # broadcast along partitions: ones[1,128]^T @ rz16[1,QG] (bf16, 213ns)
        ps_rzb = ps.tile([128, QG], F32, tag="s", name="ps_rzb")
        nc.tensor.matmul(ps_rzb[:], ones_r[:], rz16[:], start=True, stop=True)
        rzb[g] = smallp.tile([128, QG], F32, tag="rzb", name=f"rzb{g}")
        nc.vector.tensor_copy(rzb[g][:], ps_rzb[:])

    def emit_pv(g, cb, j, start, stop):
        nc.tensor.matmul(
            ps_o[(g, cb)][:],
            mvT[:, j, cb * 128:(cb + 1) * 128],
            P[g][:, j, :],
            start=start, stop=stop,
        )

    def emit_out(g, cb):
        qsl = slice(g * QG, (g + 1) * QG)
        o_sb = obp.tile([128, QG], F32, tag="ob", name="o_sb")
        nc.vector.tensor_tensor(
            out=o_sb[:], in0=ps_o.pop((g, cb))[:], in1=rzb[g][:],
            op=mybir.AluOpType.mult,
        )
        nc.sync.dma_start(out_d[cb * 128:(cb + 1) * 128, qsl], o_sb[:])

    # ---- startup: S/exp/Z for group 0, with mv transposes for cb0 woven in
    for t in range(NM):
        if t % 2 == 0:
            emit_s(0, t)
            emit_s(0, t + 1)
        if t % 4 == 0 and t >= 4:
            emit_z_quad(0, t // 4 - 1)
        if t >= 24:
            emit_transpose_quad(0, t - 24)

    # ---- flat slot loop: phases g = 0..7 (+ drain slots)
    # chain (g, cb) occupies flat slots 32g+8cb .. 32g+8cb+15, 2 matmuls/slot
    for T in range(8 * 32 + 8):
        g, t = divmod(T, 32)
        # Z tail (last quad) of the stream feeding phase g, then rz combine
        if t == 0 and g <= 7:
            emit_z_quad(g, NM // 4 - 1)
            emit_rz(g)
        # PV chains active this slot
        for cb in range(NCB):
            # chain (g, cb): local slot k = t - 8cb in [0, 16)
            k = t - 8 * cb
            if 0 <= k < 16 and g <= 7:
                if k == 0:
                    ps_o[(g, cb)] = ps.tile(
                        [128, QG], F32, tag="o", bufs=4, name=f"ps_o{g}_{cb}"
                    )
                emit_pv(g, cb, 2 * k, start=(k == 0), stop=False)
                emit_pv(g, cb, 2 * k + 1, start=False, stop=(k == 15))
                if k == 15:
                    emit_out(g, cb)
            # crossover: chain (g-1, cb3) continues into slots 0..7 of phase g
            if cb == 3 and t < 8 and g >= 1 and g - 1 <= 7:
                k2 = t + 8
                if k2 < 16:
                    emit_pv(g - 1, 3, 2 * k2, start=False, stop=False)
                    emit_pv(g - 1, 3, 2 * k2 + 1, start=False, stop=(k2 == 15))
                    if k2 == 15:
                        emit_out(g - 1, 3)
        # transposes for cb1..3 woven into phase 0 (one quad per slot),
        # offset +4 slots so each waits only on its own mv DMA
        if g == 0 and 4 <= t < 28:
            emit_transpose_quad(1 + (t - 4) // 8, (t - 4) % 8)
        # next group's S/exp/Z stream (S in row-half pairs, Z in quads)
        if g + 1 <= 7 and t < NM:
            if t % 2 == 0:
                emit_s(g + 1, t)
                emit_s(g + 1, t + 1)
            if t % 4 == 0 and t >= 4:
                emit_z_quad(g + 1, t // 4 - 1)


_prog_cache = {}


def _get_program(repeat: int = 1):
    if repeat not in _prog_cache:
        _prog_cache[repeat] = build_program(repeat)
    return _prog_cache[repeat]


def run(inputs, **spmd_kwargs):
    from concourse.bass_utils import run_bass_kernel_spmd

    Mk = np.ascontiguousarray(np.asarray(inputs["Mk"], dtype=np.float32))
    Qk = np.ascontiguousarray(np.asarray(inputs["Qk"], dtype=np.float32))
    mv = np.ascontiguousarray(np.asarray(inputs["mv"], dtype=np.float32))
    assert Mk.shape == (B, CK, H, W) and Qk.shape == (B, CK, H, W)
    assert mv.shape == (B, CV, H, W)

    in_maps = [
        {
            "Mk": Mk[b].reshape(CK, HW),
            "Qk": Qk[b].reshape(CK, HW),
            "mv": mv[b].reshape(CV, HW),
        }
        for b in range(B)
    ]
    nc = _get_program()
    res = run_bass_kernel_spmd(nc, in_maps, list(range(B)), **spmd_kwargs)
    out = np.stack([res.results[b]["out"] for b in range(B)])
    return out.reshape(B, CV, H, W).astype(np.float32), res


def kernel(**inputs) -> np.ndarray:
    out, _ = run(inputs)
    return out


# revision 6
# speedup vs baseline: 246.9128x; 1.0128x over previous
"""Bass/Trainium2 kernel for nn_AttentionMemory (scatter_memory), v2.

Reference computation (per batch b):
    S   = Mk^T @ Qk * (1/sqrt(CK))     # [HW, HW]
    P   = softmax(S, axis=memory)      # softmax over the m (row) axis
    out = mv @ P                       # [CV, HW]

Sharding: B=8 batches, one batch per NeuronCore (pure data parallel).

v2 schedule: fine-grained slot interleave. The S/exp/Z stream for group
g+1 is woven between the PV accumulation matmuls of group g so the
Activation engine (exp, 612 ns/tile) runs concurrently with PE instead of
gating a separate S phase. PV chains are staggered across "flat slots"
(chain (g,cb) occupies flat slots 32g+8cb .. +15 at 2 matmuls/slot) so
PSUM drains + out-multiplies spread out instead of bunching at group
boundaries. Z colsums ride in distinct PE column groups (tile_position)
and S matmuls in the two K=64 row halves, which run concurrently on HW.
rz broadcast moved to the idle GPSIMD engine (partition_broadcast).
"""

import numpy as np

import concourse.bass as bass
import concourse.mybir as mybir
import concourse.tile as tile
from concourse.masks import make_identity
from bass_rust import ScopedClock

B, CK, CV, H, W = 8, 64, 512, 64, 64
HW = H * W            # 4096
QG = 512              # q-group width (one PSUM bank of fp32)
NQ = HW // QG         # 8 q-groups
NM = HW // 128        # 32 m-chunks
NCB = CV // 128       # 4 c-blocks
SCALE = 1.0 / 8.0     # 1/sqrt(CK)

F32 = mybir.dt.float32
FP16 = mybir.dt.float16
BF16 = mybir.dt.bfloat16


class FixedTileContext(tile.TileContext):
    """Splits multi-wait sync_infos: this walrus accepts at most one sync
    wait per regular instruction (two on InstEventSemaphore). Extra waits
    move onto same-engine InstNoOp carriers inserted just before."""

    def _split_multi_waits(self, ordered):
        nc = self.nc
        for bb_name, insts in list(ordered.items()):
            new_insts = []
            changed = False
            for inst in insts:
                si = getattr(inst, "sync_info", None)
                waits = list(si.on_wait) if (si is not None and si.on_wait) else []
                limit = 2 if isinstance(inst, mybir.InstEventSemaphore) else 1
                if len(waits) > limit:
                    changed = True
                    for w in waits[limit:]:
                        new_insts.append(
                            mybir.InstNoOp(
                                name=nc.get_next_instruction_name(),
                                sync_info=mybir.SyncInfo(on_wait=[w], on_update=[]),
                                bass_nofuse=True,
                                engine=inst.engine,
                            )
                        )
                    inst.sync_info = mybir.SyncInfo(
                        on_wait=waits[:limit], on_update=list(si.on_update or [])
                    )
                new_insts.append(inst)
            if changed:
                ordered[bb_name] = new_insts

    def _lower_ordered_insts(self, ordered):
        self._split_multi_waits(ordered)
        return super()._lower_ordered_insts(ordered)

    def _drain_and_barrier(self, tick_clock, wait_clock):
        nc = self.nc
        drain_inst = nc.sync.drain()
        wait_clock.add_sem_waits(
            drain_inst.ins, ScopedClock({None: tick_clock.global_clock})
        )
        si = drain_inst.ins.sync_info
        waits = list(si.on_wait or []) if si is not None else []
        if len(waits) > 1:
            drain_inst.ins.sync_info = mybir.SyncInfo(
                on_wait=[waits[0]], on_update=list(si.on_update or [])
            )
            for w in waits[1:]:
                d2 = nc.sync.drain()
                d2.ins.sync_info = mybir.SyncInfo(on_wait=[w], on_update=[])
        nc.all_engine_barrier()
        assert self.sems is not None
        popped = nc._tile_sem_poison_stack.pop()
        assert popped is self._sem_poison
        nc.clear_and_free_semaphores(list(self.sems.allocated().values()))
        nc.all_engine_barrier()


def build_program(repeat: int = 1) -> bass.Bass:
    nc = bass.Bass()
    mk_d = nc.dram_tensor("Mk", [CK, HW], F32, kind="ExternalInput")
    qk_d = nc.dram_tensor("Qk", [CK, HW], F32, kind="ExternalInput")
    mv_d = nc.dram_tensor("mv", [CV, HW], F32, kind="ExternalInput")
    out_d = nc.dram_tensor("out", [CV, HW], F32, kind="ExternalOutput")

    with FixedTileContext(nc) as tc:
        with (
            tc.tile_pool(name="consts", bufs=1) as consts,
            tc.tile_pool(name="stage", bufs=2) as stage,
            tc.tile_pool(name="inp16", bufs=1) as inp16,
            tc.tile_pool(name="mvtp", bufs=1) as mvtp,
            tc.tile_pool(name="pp", bufs=2) as pp,
            tc.tile_pool(name="obp", bufs=2) as obp,
            tc.tile_pool(name="smallp", bufs=2) as smallp,
            tc.tile_pool(name="ps", bufs=2, space="PSUM") as ps,
        ):
            identity = consts.tile([128, 128], F32)
            make_identity(nc, identity[:])

            ones_h = consts.tile([128, 1], BF16)
            nc.gpsimd.memset(ones_h[:], 1.0)
            ones_r = consts.tile([1, 128], BF16)
            nc.gpsimd.memset(ones_r[:], 1.0)

            for _rep in range(repeat):
                emit_body(nc, tc, stage, inp16, mvtp, pp, obp, smallp, ps,
                          identity, ones_h, ones_r, mk_d, qk_d, mv_d, out_d)
    return nc


def emit_body(nc, tc, stage, inp16, mvtp, pp, obp, smallp, ps,
              identity, ones_h, ones_r, mk_d, qk_d, mv_d, out_d):
    # ---- input load + cast to fp16, duplicated into both partition halves
    # (ch0 via double-DMA for latency; ch1-3 single-DMA + dup casts to save
    # DMA bandwidth for the mv loads). DMA order interleaves mv so every
    # consumer's data arrives just in time under aggregate-bandwidth limits.
    mk16 = inp16.tile([128, HW], FP16)
    qk16 = inp16.tile([128, HW], FP16)
    NCH = 4
    CW = HW // NCH
    mv_sb = []

    def emit_mv_dma(cb):
        t = stage.tile([128, HW], F32, tag="mv", name=f"mv_sb{cb}")
        nc.sync.dma_start(t[:], mv_d[cb * 128:(cb + 1) * 128, :])
        mv_sb.append(t)

    for ch in range(NCH):
        csl = slice(ch * CW, (ch + 1) * CW)
        for src_d, dst in ((mk_d, mk16), (qk_d, qk16)):
            if ch == 0:
                st = stage.tile([128, CW], F32, tag="mkqk")
                nc.sync.dma_start(st[:CK, :], src_d[:, csl])
                nc.sync.dma_start(st[CK:, :], src_d[:, csl])
                nc.vector.tensor_copy(dst[:, csl], st[:])
            else:
                st = stage.tile([64, CW], F32, tag="mkqk1")
                nc.sync.dma_start(st[:], src_d[:, csl])
                nc.vector.tensor_copy(dst[:CK, csl], st[:])
                nc.gpsimd.tensor_copy(dst[CK:, csl], st[:])
        if ch == 1:
            emit_mv_dma(0)
        elif ch == 3:
            for cb in range(1, NCB):
                emit_mv_dma(cb)

    # mvT[p, j, c] = mv[c, j*128+p], bf16 (PV stationary operand)
    mvT = mvtp.tile([128, NM, CV], BF16)

    P = [None] * NQ     # P[g]: [128, NM, QG] bf16, unnormalized exp
    ps_z = [None] * NQ  # Z colsum accumulators (4 col-group partials)
    rzb = [None] * NQ   # broadcast 1/Z rows
    ps_o = {}           # (g, cb) -> PV accumulation PSUM tile

    def emit_transpose_quad(cb, q):
        """Transpose m-chunks j=4q..4q+3 of mv c-block cb: 4 PE transposes
        into one PSUM tile, one DVE copy out (keeps the shared s-ring at
        ~2 allocs/slot)."""
        ps_t = ps.tile([128, QG], F32, tag="s", name="ps_t")
        for jj in range(4):
            j4 = 4 * q + jj
            nc.tensor.transpose(
                ps_t[:, jj * 128:(jj + 1) * 128],
                mv_sb[cb][:, j4 * 128:(j4 + 1) * 128], identity[:]
            )
        nc.vector.tensor_copy(
            mvT[:, 4 * q:4 * q + 4, cb * 128:(cb + 1) * 128],
            ps_t.rearrange("p (j c) -> p j c", j=4),
        )

    def emit_s(g, j):
        """One S matmul + exp for (g, j). Allocates P[g]/ps_z[g] on j==0."""
        if j == 0:
            P[g] = pp.tile([128, NM, QG], BF16, tag="P", name=f"P{g}")
            ps_z[g] = ps.tile([128, QG], F32, tag="z", name=f"ps_z{g}")
        qsl = slice(g * QG, (g + 1) * QG)
        half = j % 2
        ksl = slice(half * CK, half * CK + CK)
        ps_sj = ps.tile([128, QG], F32, tag="s", name="ps_s")
        nc.tensor.matmul(
            ps_sj[:], mk16[ksl, j * 128:(j + 1) * 128], qk16[ksl, qsl],
            start=True, stop=True,
        )
        nc.scalar.activation(
            P[g][:, j, :], ps_sj[:],
            mybir.ActivationFunctionType.Exp, scale=SCALE,
        )

    def emit_z_quad(g, a):
        """Z colsum chunks j=4a..4a+3 for group g, emitted back-to-back so
        the 4 column-group chains run concurrently on the PE array."""
        for c in range(4):
            nc.tensor.matmul(
                ps_z[g][32 * c:32 * c + 1, :], ones_h[:], P[g][:, 4 * a + c, :],
                start=(a == 0), stop=(a == NM // 4 - 1),
                tile_position=(0, 32 * c),
            )

    def emit_rz(g):
        """Combine Z partials -> reciprocal -> broadcast (DVE + GPSIMD)."""
        za = smallp.tile([1, QG], F32, tag="zt", name="za")
        nc.vector.tensor_copy(za[:], ps_z[g][0:1, :])
        zb = smallp.tile([1, QG], F32, tag="zt", name="zb")
        nc.vector.tensor_tensor(
            out=zb[:], in0=za[:], in1=ps_z[g][32:33, :], op=mybir.AluOpType.add
        )
        zc = smallp.tile([1, QG], F32, tag="zt", name="zc")
        nc.vector.tensor_tensor(
            out=zc[:], in0=zb[:], in1=ps_z[g][64:65, :], op=mybir.AluOpType.add
        )
        zs = smallp.tile([1, QG], F32, tag="zt", name="zs")
        nc.vector.tensor_tensor(
            out=zs[:], in0=zc[:], in1=ps_z[g][96:97, :], op=mybir.AluOpType.add
        )
        rz = smallp.tile([1, QG], F32, tag="rz", name="rz")
        nc.vector.reciprocal(rz[:], zs[:])
        rz16 = smallp.tile([1, QG], BF16, tag="rz16", name="rz16")
        nc.vector.tensor_copy(rz16[:], rz[:])
        # broadcast along partitions: ones[1,128]^T @ rz16[1,QG] (bf16, 213ns)
        ps_rzb = ps.tile([128, QG], F32, tag="s", name="ps_rzb")
        nc.tensor.matmul(ps_rzb[:], ones_r[:], rz16[:], start=True, stop=True)
        rzb[g] = smallp.tile([128, QG], F32, tag="rzb", name=f"rzb{g}")
        nc.vector.tensor_copy(rzb[g][:], ps_rzb[:])

    def emit_pv(g, cb, j, start, stop):
        nc.tensor.matmul(
            ps_o[(g, cb)][:],
            mvT[:, j, cb * 128:(cb + 1) * 128],
            P[g][:, j, :],
            start=start, stop=stop,
        )

    def emit_out(g, cb):
        qsl = slice(g * QG, (g + 1) * QG)
        o_sb = obp.tile([128, QG], F32, tag="ob", name="o_sb")
        nc.vector.tensor_tensor(
            out=o_sb[:], in0=ps_o.pop((g, cb))[:], in1=rzb[g][:],
            op=mybir.AluOpType.mult,
        )
        nc.sync.dma_start(out_d[cb * 128:(cb + 1) * 128, qsl], o_sb[:])

    def chain_emits(s, t):
        """PV chain work due at slot t of stream s. Chain (g, cb) occupies
        stream-g slots 18+8cb .. 31 and stream-(g+1) slots 0 .. 8cb+1."""
        for cb in range(NCB):
            for g, k in ((s, t - 18 - 8 * cb), (s - 1, 32 + t - 18 - 8 * cb)):
                if 0 <= g < NQ and 0 <= k < 16:
                    if k == 0:
                        ps_o[(g, cb)] = ps.tile(
                            [128, QG], F32, tag="o", bufs=4, name=f"ps_o{g}_{cb}"
                        )
                    emit_pv(g, cb, 2 * k, start=(k == 0), stop=False)
                    emit_pv(g, cb, 2 * k + 1, start=False, stop=(k == 15))
                    if k == 15:
                        emit_out(g, cb)

    # ---- startup (stream 0): S/exp/Z for group 0, cb0/cb1 transposes,
    # and the head of group 0's PV chains
    for t in range(NM):
        if t % 2 == 0:
            emit_s(0, t)
            emit_s(0, t + 1)
        if t % 4 == 0 and t >= 4:
            emit_z_quad(0, t // 4 - 1)
        if 14 <= t < 22:
            emit_transpose_quad(0, t - 14)
        if t >= 24:
            emit_transpose_quad(1, t - 24)
        chain_emits(0, t)

    # ---- phases p = 0..7 (stream s = p+1 slots)
    for T in range(8 * 32):
        p, t = divmod(T, 32)
        if t == 0:
            emit_z_quad(p, NM // 4 - 1)
            emit_rz(p)
        if p == 0 and 2 <= t < 10:
            emit_transpose_quad(2, t - 2)
        if p == 0 and 10 <= t < 18:
            emit_transpose_quad(3, t - 10)
        chain_emits(p + 1, t)
        if p + 1 <= 7 and t < NM:
            if t % 2 == 0:
                emit_s(p + 1, t)
                emit_s(p + 1, t + 1)
            if t % 4 == 0 and t >= 4:
                emit_z_quad(p + 1, t // 4 - 1)


_prog_cache = {}


def _get_program(repeat: int = 1):
    if repeat not in _prog_cache:
        _prog_cache[repeat] = build_program(repeat)
    return _prog_cache[repeat]


def run(inputs, **spmd_kwargs):
    from concourse.bass_utils import run_bass_kernel_spmd

    Mk = np.ascontiguousarray(np.asarray(inputs["Mk"], dtype=np.float32))
    Qk = np.ascontiguousarray(np.asarray(inputs["Qk"], dtype=np.float32))
    mv = np.ascontiguousarray(np.asarray(inputs["mv"], dtype=np.float32))
    assert Mk.shape == (B, CK, H, W) and Qk.shape == (B, CK, H, W)
    assert mv.shape == (B, CV, H, W)

    in_maps = [
        {
            "Mk": Mk[b].reshape(CK, HW),
            "Qk": Qk[b].reshape(CK, HW),
            "mv": mv[b].reshape(CV, HW),
        }
        for b in range(B)
    ]
    nc = _get_program()
    res = run_bass_kernel_spmd(nc, in_maps, list(range(B)), **spmd_kwargs)
    out = np.stack([res.results[b]["out"] for b in range(B)])
    return out.reshape(B, CV, H, W).astype(np.float32), res


def kernel(**inputs) -> np.ndarray:
    out, _ = run(inputs)
    return out


# revision 7
# speedup vs baseline: 250.7274x; 1.0154x over previous
"""Bass/Trainium2 kernel for nn_AttentionMemory (scatter_memory), v2.

Reference computation (per batch b):
    S   = Mk^T @ Qk * (1/sqrt(CK))     # [HW, HW]
    P   = softmax(S, axis=memory)      # softmax over the m (row) axis
    out = mv @ P                       # [CV, HW]

Sharding: B=8 batches, one batch per NeuronCore (pure data parallel).

v2 schedule: fine-grained slot interleave. The S/exp/Z stream for group
g+1 is woven between the PV accumulation matmuls of group g so the
Activation engine (exp, 612 ns/tile) runs concurrently with PE instead of
gating a separate S phase. PV chains are staggered across "flat slots"
(chain (g,cb) occupies flat slots 32g+8cb .. +15 at 2 matmuls/slot) so
PSUM drains + out-multiplies spread out instead of bunching at group
boundaries. Z colsums ride in distinct PE column groups (tile_position)
and S matmuls in the two K=64 row halves, which run concurrently on HW.
rz broadcast moved to the idle GPSIMD engine (partition_broadcast).
"""

import numpy as np

import concourse.bass as bass
import concourse.mybir as mybir
import concourse.tile as tile
from concourse.masks import make_identity
from bass_rust import ScopedClock

B, CK, CV, H, W = 8, 64, 512, 64, 64
HW = H * W            # 4096
QG = 512              # q-group width (one PSUM bank of fp32)
NQ = HW // QG         # 8 q-groups
NM = HW // 128        # 32 m-chunks
NCB = CV // 128       # 4 c-blocks
SCALE = 1.0 / 8.0     # 1/sqrt(CK)

F32 = mybir.dt.float32
FP16 = mybir.dt.float16
BF16 = mybir.dt.bfloat16


class FixedTileContext(tile.TileContext):
    """Splits multi-wait sync_infos: this walrus accepts at most one sync
    wait per regular instruction (two on InstEventSemaphore). Extra waits
    move onto same-engine InstNoOp carriers inserted just before."""

    def _split_multi_waits(self, ordered):
        nc = self.nc
        for bb_name, insts in list(ordered.items()):
            new_insts = []
            changed = False
            for inst in insts:
                si = getattr(inst, "sync_info", None)
                waits = list(si.on_wait) if (si is not None and si.on_wait) else []
                limit = 2 if isinstance(inst, mybir.InstEventSemaphore) else 1
                if len(waits) > limit:
                    changed = True
                    for w in waits[limit:]:
                        new_insts.append(
                            mybir.InstNoOp(
                                name=nc.get_next_instruction_name(),
                                sync_info=mybir.SyncInfo(on_wait=[w], on_update=[]),
                                bass_nofuse=True,
                                engine=inst.engine,
                            )
                        )
                    inst.sync_info = mybir.SyncInfo(
                        on_wait=waits[:limit], on_update=list(si.on_update or [])
                    )
                new_insts.append(inst)
            if changed:
                ordered[bb_name] = new_insts

    def _lower_ordered_insts(self, ordered):
        self._split_multi_waits(ordered)
        return super()._lower_ordered_insts(ordered)

    def _drain_and_barrier(self, tick_clock, wait_clock):
        nc = self.nc
        drain_inst = nc.sync.drain()
        wait_clock.add_sem_waits(
            drain_inst.ins, ScopedClock({None: tick_clock.global_clock})
        )
        si = drain_inst.ins.sync_info
        waits = list(si.on_wait or []) if si is not None else []
        if len(waits) > 1:
            drain_inst.ins.sync_info = mybir.SyncInfo(
                on_wait=[waits[0]], on_update=list(si.on_update or [])
            )
            for w in waits[1:]:
                d2 = nc.sync.drain()
                d2.ins.sync_info = mybir.SyncInfo(on_wait=[w], on_update=[])
        nc.all_engine_barrier()
        assert self.sems is not None
        popped = nc._tile_sem_poison_stack.pop()
        assert popped is self._sem_poison
        nc.clear_and_free_semaphores(list(self.sems.allocated().values()))
        nc.all_engine_barrier()


def build_program(repeat: int = 1) -> bass.Bass:
    nc = bass.Bass()
    mk_d = nc.dram_tensor("Mk", [CK, HW], F32, kind="ExternalInput")
    qk_d = nc.dram_tensor("Qk", [CK, HW], F32, kind="ExternalInput")
    mv_d = nc.dram_tensor("mv", [CV, HW], F32, kind="ExternalInput")
    out_d = nc.dram_tensor("out", [CV, HW], F32, kind="ExternalOutput")

    with FixedTileContext(nc) as tc:
        with (
            tc.tile_pool(name="consts", bufs=1) as consts,
            tc.tile_pool(name="stage", bufs=2) as stage,
            tc.tile_pool(name="inp16", bufs=1) as inp16,
            tc.tile_pool(name="mvtp", bufs=1) as mvtp,
            tc.tile_pool(name="pp", bufs=2) as pp,
            tc.tile_pool(name="obp", bufs=2) as obp,
            tc.tile_pool(name="smallp", bufs=2) as smallp,
            tc.tile_pool(name="ps", bufs=2, space="PSUM") as ps,
        ):
            identity = consts.tile([128, 128], F32)
            make_identity(nc, identity[:])
            ident16 = consts.tile([128, 128], BF16)
            nc.vector.tensor_copy(ident16[:], identity[:])

            ones_h = consts.tile([128, 1], BF16)
            nc.gpsimd.memset(ones_h[:], 1.0)
            ones_r = consts.tile([1, 128], BF16)
            nc.gpsimd.memset(ones_r[:], 1.0)

            for _rep in range(repeat):
                emit_body(nc, tc, stage, inp16, mvtp, pp, obp, smallp, ps,
                          ident16, ones_h, ones_r, mk_d, qk_d, mv_d, out_d)
    return nc


def emit_body(nc, tc, stage, inp16, mvtp, pp, obp, smallp, ps,
              ident16, ones_h, ones_r, mk_d, qk_d, mv_d, out_d):
    # ---- input load + cast to fp16, duplicated into both partition halves
    # (ch0 via double-DMA for latency; ch1-3 single-DMA + dup casts to save
    # DMA bandwidth for the mv loads). DMA order interleaves mv so every
    # consumer's data arrives just in time under aggregate-bandwidth limits.
    mk16 = inp16.tile([128, HW], FP16)
    qk16 = inp16.tile([128, HW], FP16)
    NCH = 4
    CW = HW // NCH
    mv_sb = []

    def emit_mv_dma(cb):
        t = stage.tile([128, HW], F32, tag="mv", name=f"mv_sb{cb}")
        nc.sync.dma_start(t[:], mv_d[cb * 128:(cb + 1) * 128, :])
        mv_sb.append(t)

    for ch in range(NCH):
        csl = slice(ch * CW, (ch + 1) * CW)
        for src_d, dst in ((mk_d, mk16), (qk_d, qk16)):
            if ch == 0:
                st = stage.tile([128, CW], F32, tag="mkqk")
                nc.sync.dma_start(st[:CK, :], src_d[:, csl])
                nc.sync.dma_start(st[CK:, :], src_d[:, csl])
                nc.vector.tensor_copy(dst[:, csl], st[:])
            else:
                st = stage.tile([64, CW], F32, tag="mkqk1")
                nc.sync.dma_start(st[:], src_d[:, csl])
                nc.vector.tensor_copy(dst[:CK, csl], st[:])
                nc.gpsimd.tensor_copy(dst[CK:, csl], st[:])
        if ch == 1:
            emit_mv_dma(0)
        elif ch == 3:
            for cb in range(1, NCB):
                emit_mv_dma(cb)

    # mvT[p, j, c] = mv[c, j*128+p], bf16 (PV stationary operand)
    mvT = mvtp.tile([128, NM, CV], BF16)

    P = [None] * NQ     # P[g]: [128, NM, QG] bf16, unnormalized exp
    ps_z = [None] * NQ  # Z colsum accumulators (4 col-group partials)
    rzb = [None] * NQ   # broadcast 1/Z rows
    ps_o = {}           # (g, cb) -> PV accumulation PSUM tile

    def emit_transpose_quad(cb, q):
        """Transpose m-chunks j=4q..4q+3 of mv c-block cb: 4 PE transposes
        into one PSUM tile, one DVE copy out (keeps the shared s-ring at
        ~2 allocs/slot)."""
        mq = stage.tile([128, QG], BF16, tag="mq", bufs=4, name="mq")
        nc.vector.tensor_copy(mq[:], mv_sb[cb][:, QG * q:QG * (q + 1)])
        ps_t = ps.tile([128, QG], BF16, tag="s", name="ps_t")
        for jj in range(4):
            nc.tensor.transpose(
                ps_t[:, jj * 128:(jj + 1) * 128],
                mq[:, jj * 128:(jj + 1) * 128], ident16[:]
            )
        nc.vector.tensor_copy(
            mvT[:, 4 * q:4 * q + 4, cb * 128:(cb + 1) * 128],
            ps_t.rearrange("p (j c) -> p j c", j=4),
        )

    def emit_s(g, j):
        """One S matmul + exp for (g, j). Allocates P[g]/ps_z[g] on j==0."""
        if j == 0:
            P[g] = pp.tile([128, NM, QG], BF16, tag="P", name=f"P{g}")
            ps_z[g] = ps.tile([128, QG], F32, tag="z", name=f"ps_z{g}")
        qsl = slice(g * QG, (g + 1) * QG)
        half = j % 2
        ksl = slice(half * CK, half * CK + CK)
        ps_sj = ps.tile([128, QG], F32, tag="s", name="ps_s")
        nc.tensor.matmul(
            ps_sj[:], mk16[ksl, j * 128:(j + 1) * 128], qk16[ksl, qsl],
            start=True, stop=True,
        )
        nc.scalar.activation(
            P[g][:, j, :], ps_sj[:],
            mybir.ActivationFunctionType.Exp, scale=SCALE,
        )

    def emit_z_quad(g, a):
        """Z colsum chunks j=4a..4a+3 for group g, emitted back-to-back so
        the 4 column-group chains run concurrently on the PE array."""
        for c in range(4):
            nc.tensor.matmul(
                ps_z[g][32 * c:32 * c + 1, :], ones_h[:], P[g][:, 4 * a + c, :],
                start=(a == 0), stop=(a == NM // 4 - 1),
                tile_position=(0, 32 * c),
            )

    def emit_rz(g):
        """Combine Z partials -> reciprocal -> broadcast (DVE + GPSIMD)."""
        za = smallp.tile([1, QG], F32, tag="zt", name="za")
        nc.vector.tensor_copy(za[:], ps_z[g][0:1, :])
        zb = smallp.tile([1, QG], F32, tag="zt", name="zb")
        nc.vector.tensor_tensor(
            out=zb[:], in0=za[:], in1=ps_z[g][32:33, :], op=mybir.AluOpType.add
        )
        zc = smallp.tile([1, QG], F32, tag="zt", name="zc")
        nc.vector.tensor_tensor(
            out=zc[:], in0=zb[:], in1=ps_z[g][64:65, :], op=mybir.AluOpType.add
        )
        zs = smallp.tile([1, QG], F32, tag="zt", name="zs")
        nc.vector.tensor_tensor(
            out=zs[:], in0=zc[:], in1=ps_z[g][96:97, :], op=mybir.AluOpType.add
        )
        rz = smallp.tile([1, QG], F32, tag="rz", name="rz")
        nc.vector.reciprocal(rz[:], zs[:])
        rz16 = smallp.tile([1, QG], BF16, tag="rz16", name="rz16")
        nc.vector.tensor_copy(rz16[:], rz[:])
        # broadcast along partitions: ones[1,128]^T @ rz16[1,QG] (bf16, 213ns)
        ps_rzb = ps.tile([128, QG], F32, tag="s", name="ps_rzb")
        nc.tensor.matmul(ps_rzb[:], ones_r[:], rz16[:], start=True, stop=True)
        rzb[g] = smallp.tile([128, QG], F32, tag="rzb", name=f"rzb{g}")
        nc.vector.tensor_copy(rzb[g][:], ps_rzb[:])

    def emit_pv(g, cb, j, start, stop):
        nc.tensor.matmul(
            ps_o[(g, cb)][:],
            mvT[:, j, cb * 128:(cb + 1) * 128],
            P[g][:, j, :],
            start=start, stop=stop,
        )

    def emit_out(g, cb):
        qsl = slice(g * QG, (g + 1) * QG)
        o_sb = obp.tile([128, QG], F32, tag="ob", name="o_sb")
        nc.vector.tensor_tensor(
            out=o_sb[:], in0=ps_o.pop((g, cb))[:], in1=rzb[g][:],
            op=mybir.AluOpType.mult,
        )
        nc.sync.dma_start(out_d[cb * 128:(cb + 1) * 128, qsl], o_sb[:])

    def chain_emits(s, t):
        """PV chain work due at slot t of stream s. Chain (g, cb) occupies
        stream-g slots 18+8cb .. 31 and stream-(g+1) slots 0 .. 8cb+1."""
        for cb in range(NCB):
            for g, k in ((s, t - 18 - 8 * cb), (s - 1, 32 + t - 18 - 8 * cb)):
                if 0 <= g < NQ and 0 <= k < 16:
                    if k == 0:
                        ps_o[(g, cb)] = ps.tile(
                            [128, QG], F32, tag="o", bufs=4, name=f"ps_o{g}_{cb}"
                        )
                    emit_pv(g, cb, 2 * k, start=(k == 0), stop=False)
                    emit_pv(g, cb, 2 * k + 1, start=False, stop=(k == 15))
                    if k == 15:
                        emit_out(g, cb)

    # ---- startup (stream 0): S/exp/Z for group 0, cb0/cb1 transposes,
    # and the head of group 0's PV chains
    for t in range(NM):
        if t % 2 == 0:
            emit_s(0, t)
            emit_s(0, t + 1)
        if t % 4 == 0 and t >= 4:
            emit_z_quad(0, t // 4 - 1)
        if 14 <= t < 22:
            emit_transpose_quad(0, t - 14)
        if t >= 24:
            emit_transpose_quad(1, t - 24)
        chain_emits(0, t)

    # ---- phases p = 0..7 (stream s = p+1 slots)
    for T in range(8 * 32):
        p, t = divmod(T, 32)
        if t == 0:
            emit_z_quad(p, NM // 4 - 1)
            emit_rz(p)
        if p == 0 and 2 <= t < 10:
            emit_transpose_quad(2, t - 2)
        if p == 0 and 10 <= t < 18:
            emit_transpose_quad(3, t - 10)
        chain_emits(p + 1, t)
        if p + 1 <= 7 and t < NM:
            if t % 2 == 0:
                emit_s(p + 1, t)
                emit_s(p + 1, t + 1)
            if t % 4 == 0 and t >= 4:
                emit_z_quad(p + 1, t // 4 - 1)


_prog_cache = {}


def _get_program(repeat: int = 1):
    if repeat not in _prog_cache:
        _prog_cache[repeat] = build_program(repeat)
    return _prog_cache[repeat]


def run(inputs, **spmd_kwargs):
    from concourse.bass_utils import run_bass_kernel_spmd

    Mk = np.ascontiguousarray(np.asarray(inputs["Mk"], dtype=np.float32))
    Qk = np.ascontiguousarray(np.asarray(inputs["Qk"], dtype=np.float32))
    mv = np.ascontiguousarray(np.asarray(inputs["mv"], dtype=np.float32))
    assert Mk.shape == (B, CK, H, W) and Qk.shape == (B, CK, H, W)
    assert mv.shape == (B, CV, H, W)

    in_maps = [
        {
            "Mk": Mk[b].reshape(CK, HW),
            "Qk": Qk[b].reshape(CK, HW),
            "mv": mv[b].reshape(CV, HW),
        }
        for b in range(B)
    ]
    nc = _get_program()
    res = run_bass_kernel_spmd(nc, in_maps, list(range(B)), **spmd_kwargs)
    out = np.stack([res.results[b]["out"] for b in range(B)])
    return out.reshape(B, CV, H, W).astype(np.float32), res


def kernel(**inputs) -> np.ndarray:
    out, _ = run(inputs)
    return out


# revision 8
# speedup vs baseline: 251.5853x; 1.0034x over previous
"""Bass/Trainium2 kernel for nn_AttentionMemory (scatter_memory), v2.

Reference computation (per batch b):
    S   = Mk^T @ Qk * (1/sqrt(CK))     # [HW, HW]
    P   = softmax(S, axis=memory)      # softmax over the m (row) axis
    out = mv @ P                       # [CV, HW]

Sharding: B=8 batches, one batch per NeuronCore (pure data parallel).

v2 schedule: fine-grained slot interleave. The S/exp/Z stream for group
g+1 is woven between the PV accumulation matmuls of group g so the
Activation engine (exp, 612 ns/tile) runs concurrently with PE instead of
gating a separate S phase. PV chains are staggered across "flat slots"
(chain (g,cb) occupies flat slots 32g+8cb .. +15 at 2 matmuls/slot) so
PSUM drains + out-multiplies spread out instead of bunching at group
boundaries. Z colsums ride in distinct PE column groups (tile_position)
and S matmuls in the two K=64 row halves, which run concurrently on HW.
rz broadcast moved to the idle GPSIMD engine (partition_broadcast).
"""

import numpy as np

import concourse.bass as bass
import concourse.mybir as mybir
import concourse.tile as tile
from concourse.masks import make_identity
from bass_rust import ScopedClock

B, CK, CV, H, W = 8, 64, 512, 64, 64
HW = H * W            # 4096
QG = 512              # q-group width (one PSUM bank of fp32)
NQ = HW // QG         # 8 q-groups
NM = HW // 128        # 32 m-chunks
NCB = CV // 128       # 4 c-blocks
SCALE = 1.0 / 8.0     # 1/sqrt(CK)

F32 = mybir.dt.float32
FP16 = mybir.dt.float16
BF16 = mybir.dt.bfloat16


class FixedTileContext(tile.TileContext):
    """Splits multi-wait sync_infos: this walrus accepts at most one sync
    wait per regular instruction (two on InstEventSemaphore). Extra waits
    move onto same-engine InstNoOp carriers inserted just before."""

    def _split_multi_waits(self, ordered):
        nc = self.nc
        for bb_name, insts in list(ordered.items()):
            new_insts = []
            changed = False
            for inst in insts:
                si = getattr(inst, "sync_info", None)
                waits = list(si.on_wait) if (si is not None and si.on_wait) else []
                limit = 2 if isinstance(inst, mybir.InstEventSemaphore) else 1
                if len(waits) > limit:
                    changed = True
                    for w in waits[limit:]:
                        new_insts.append(
                            mybir.InstNoOp(
                                name=nc.get_next_instruction_name(),
                                sync_info=mybir.SyncInfo(on_wait=[w], on_update=[]),
                                bass_nofuse=True,
                                engine=inst.engine,
                            )
                        )
                    inst.sync_info = mybir.SyncInfo(
                        on_wait=waits[:limit], on_update=list(si.on_update or [])
                    )
                new_insts.append(inst)
            if changed:
                ordered[bb_name] = new_insts

    def _lower_ordered_insts(self, ordered):
        self._split_multi_waits(ordered)
        return super()._lower_ordered_insts(ordered)

    def _drain_and_barrier(self, tick_clock, wait_clock):
        nc = self.nc
        drain_inst = nc.sync.drain()
        wait_clock.add_sem_waits(
            drain_inst.ins, ScopedClock({None: tick_clock.global_clock})
        )
        si = drain_inst.ins.sync_info
        waits = list(si.on_wait or []) if si is not None else []
        if len(waits) > 1:
            drain_inst.ins.sync_info = mybir.SyncInfo(
                on_wait=[waits[0]], on_update=list(si.on_update or [])
            )
            for w in waits[1:]:
                d2 = nc.sync.drain()
                d2.ins.sync_info = mybir.SyncInfo(on_wait=[w], on_update=[])
        nc.all_engine_barrier()
        assert self.sems is not None
        popped = nc._tile_sem_poison_stack.pop()
        assert popped is self._sem_poison
        nc.clear_and_free_semaphores(list(self.sems.allocated().values()))
        nc.all_engine_barrier()


def build_program(repeat: int = 1) -> bass.Bass:
    nc = bass.Bass()
    mk_d = nc.dram_tensor("Mk", [CK, HW], F32, kind="ExternalInput")
    qk_d = nc.dram_tensor("Qk", [CK, HW], F32, kind="ExternalInput")
    mv_d = nc.dram_tensor("mv", [CV, HW], F32, kind="ExternalInput")
    out_d = nc.dram_tensor("out", [CV, HW], F32, kind="ExternalOutput")

    with FixedTileContext(nc) as tc:
        with (
            tc.tile_pool(name="consts", bufs=1) as consts,
            tc.tile_pool(name="stage", bufs=2) as stage,
            tc.tile_pool(name="inp16", bufs=1) as inp16,
            tc.tile_pool(name="mvtp", bufs=1) as mvtp,
            tc.tile_pool(name="pp", bufs=2) as pp,
            tc.tile_pool(name="obp", bufs=2) as obp,
            tc.tile_pool(name="smallp", bufs=2) as smallp,
            tc.tile_pool(name="ps", bufs=2, space="PSUM") as ps,
        ):
            identity = consts.tile([128, 128], F32)
            make_identity(nc, identity[:])
            ident16 = consts.tile([128, 128], BF16)
            nc.vector.tensor_copy(ident16[:], identity[:])

            ones_h = consts.tile([128, 1], BF16)
            nc.gpsimd.memset(ones_h[:], 1.0)
            ones_r = consts.tile([1, 128], BF16)
            nc.gpsimd.memset(ones_r[:], 1.0)

            for _rep in range(repeat):
                emit_body(nc, tc, stage, inp16, mvtp, pp, obp, smallp, ps,
                          ident16, ones_h, ones_r, mk_d, qk_d, mv_d, out_d)
    return nc


def emit_body(nc, tc, stage, inp16, mvtp, pp, obp, smallp, ps,
              ident16, ones_h, ones_r, mk_d, qk_d, mv_d, out_d):
    # ---- HW warmup during the initial DMA wait (both invisible to the
    # cost-model sim, real on hardware):
    #  - dummy exp: pulls the ~1.3us activation-table load off the first
    #    real exp, which otherwise gates the S stream
    #  - dummy matmuls: keep the PE busy through the HAM activity window so
    #    the real S matmuls start at 2.4 GHz instead of the cold 1.2 GHz
    #    (PE-transposes don't count as HAM activity; matmuls do)
    warm_o = smallp.tile([128, 1], F32, tag="warm", bufs=1, name="warm_o")
    nc.scalar.activation(warm_o[:], ones_h[:],
                         mybir.ActivationFunctionType.Exp, scale=1.0)
    ps_warm = ps.tile([128, QG], F32, tag="s", name="ps_warm")
    for _ in range(60):
        nc.tensor.matmul(ps_warm[0:1, :128], ones_h[:], ident16[:],
                         start=True, stop=True)

    # ---- input load + cast to fp16, duplicated into both partition halves
    # (ch0 via double-DMA for latency; ch1-3 single-DMA + dup casts to save
    # DMA bandwidth for the mv loads). DMA order interleaves mv so every
    # consumer's data arrives just in time under aggregate-bandwidth limits.
    mk16 = inp16.tile([128, HW], FP16)
    qk16 = inp16.tile([128, HW], FP16)
    NCH = 4
    CW = HW // NCH
    mv_sb = []

    def emit_mv_dma(cb):
        t = stage.tile([128, HW], F32, tag="mv", name=f"mv_sb{cb}")
        nc.sync.dma_start(t[:], mv_d[cb * 128:(cb + 1) * 128, :])
        mv_sb.append(t)

    for ch in range(NCH):
        csl = slice(ch * CW, (ch + 1) * CW)
        for src_d, dst in ((mk_d, mk16), (qk_d, qk16)):
            if ch == 0:
                st = stage.tile([128, CW], F32, tag="mkqk")
                nc.sync.dma_start(st[:CK, :], src_d[:, csl])
                nc.sync.dma_start(st[CK:, :], src_d[:, csl])
                nc.vector.tensor_copy(dst[:, csl], st[:])
            else:
                st = stage.tile([64, CW], F32, tag="mkqk1")
                nc.sync.dma_start(st[:], src_d[:, csl])
                nc.vector.tensor_copy(dst[:CK, csl], st[:])
                nc.gpsimd.tensor_copy(dst[CK:, csl], st[:])
        if ch == 1:
            emit_mv_dma(0)
        elif ch == 3:
            for cb in range(1, NCB):
                emit_mv_dma(cb)

    # mvT[p, j, c] = mv[c, j*128+p], bf16 (PV stationary operand)
    mvT = mvtp.tile([128, NM, CV], BF16)

    P = [None] * NQ     # P[g]: [128, NM, QG] bf16, unnormalized exp
    ps_z = [None] * NQ  # Z colsum accumulators (4 col-group partials)
    rzb = [None] * NQ   # broadcast 1/Z rows
    ps_o = {}           # (g, cb) -> PV accumulation PSUM tile

    def emit_transpose_quad(cb, q):
        """Transpose m-chunks j=4q..4q+3 of mv c-block cb: 4 PE transposes
        into one PSUM tile, one DVE copy out (keeps the shared s-ring at
        ~2 allocs/slot)."""
        mq = stage.tile([128, QG], BF16, tag="mq", bufs=4, name="mq")
        nc.vector.tensor_copy(mq[:], mv_sb[cb][:, QG * q:QG * (q + 1)])
        ps_t = ps.tile([128, QG], BF16, tag="s", name="ps_t")
        for jj in range(4):
            nc.tensor.transpose(
                ps_t[:, jj * 128:(jj + 1) * 128],
                mq[:, jj * 128:(jj + 1) * 128], ident16[:]
            )
        nc.vector.tensor_copy(
            mvT[:, 4 * q:4 * q + 4, cb * 128:(cb + 1) * 128],
            ps_t.rearrange("p (j c) -> p j c", j=4),
        )

    def emit_s(g, j):
        """One S matmul + exp for (g, j). Allocates P[g]/ps_z[g] on j==0."""
        if j == 0:
            P[g] = pp.tile([128, NM, QG], BF16, tag="P", name=f"P{g}")
            ps_z[g] = ps.tile([128, QG], F32, tag="z", name=f"ps_z{g}")
        qsl = slice(g * QG, (g + 1) * QG)
        half = j % 2
        ksl = slice(half * CK, half * CK + CK)
        ps_sj = ps.tile([128, QG], F32, tag="s", name="ps_s")
        nc.tensor.matmul(
            ps_sj[:], mk16[ksl, j * 128:(j + 1) * 128], qk16[ksl, qsl],
            start=True, stop=True,
        )
        nc.scalar.activation(
            P[g][:, j, :], ps_sj[:],
            mybir.ActivationFunctionType.Exp, scale=SCALE,
        )

    def emit_z_quad(g, a):
        """Z colsum chunks j=4a..4a+3 for group g, emitted back-to-back so
        the 4 column-group chains run concurrently on the PE array."""
        for c in range(4):
            nc.tensor.matmul(
                ps_z[g][32 * c:32 * c + 1, :], ones_h[:], P[g][:, 4 * a + c, :],
                start=(a == 0), stop=(a == NM // 4 - 1),
                tile_position=(0, 32 * c),
            )

    def emit_rz(g):
        """Combine Z partials -> reciprocal -> broadcast (DVE + GPSIMD)."""
        za = smallp.tile([1, QG], F32, tag="zt", name="za")
        nc.vector.tensor_copy(za[:], ps_z[g][0:1, :])
        zb = smallp.tile([1, QG], F32, tag="zt", name="zb")
        nc.vector.tensor_tensor(
            out=zb[:], in0=za[:], in1=ps_z[g][32:33, :], op=mybir.AluOpType.add
        )
        zc = smallp.tile([1, QG], F32, tag="zt", name="zc")
        nc.vector.tensor_tensor(
            out=zc[:], in0=zb[:], in1=ps_z[g][64:65, :], op=mybir.AluOpType.add
        )
        zs = smallp.tile([1, QG], F32, tag="zt", name="zs")
        nc.vector.tensor_tensor(
            out=zs[:], in0=zc[:], in1=ps_z[g][96:97, :], op=mybir.AluOpType.add
        )
        rz = smallp.tile([1, QG], F32, tag="rz", name="rz")
        nc.vector.reciprocal(rz[:], zs[:])
        rz16 = smallp.tile([1, QG], BF16, tag="rz16", name="rz16")
        nc.vector.tensor_copy(rz16[:], rz[:])
        # broadcast along partitions: ones[1,128]^T @ rz16[1,QG] (bf16, 213ns)
        ps_rzb = ps.tile([128, QG], F32, tag="s", name="ps_rzb")
        nc.tensor.matmul(ps_rzb[:], ones_r[:], rz16[:], start=True, stop=True)
        rzb[g] = smallp.tile([128, QG], F32, tag="rzb", name=f"rzb{g}")
        nc.vector.tensor_copy(rzb[g][:], ps_rzb[:])

    def emit_pv(g, cb, j, start, stop):
        nc.tensor.matmul(
            ps_o[(g, cb)][:],
            mvT[:, j, cb * 128:(cb + 1) * 128],
            P[g][:, j, :],
            start=start, stop=stop,
        )

    def emit_out(g, cb):
        qsl = slice(g * QG, (g + 1) * QG)
        o_sb = obp.tile([128, QG], F32, tag="ob", name="o_sb")
        nc.vector.tensor_tensor(
            out=o_sb[:], in0=ps_o.pop((g, cb))[:], in1=rzb[g][:],
            op=mybir.AluOpType.mult,
        )
        nc.sync.dma_start(out_d[cb * 128:(cb + 1) * 128, qsl], o_sb[:])

    def chain_emits(s, t):
        """PV chain work due at slot t of stream s. Chain (g, cb) occupies
        stream-g slots 18+8cb .. 31 and stream-(g+1) slots 0 .. 8cb+1."""
        for cb in range(NCB):
            for g, k in ((s, t - 18 - 8 * cb), (s - 1, 32 + t - 18 - 8 * cb)):
                if 0 <= g < NQ and 0 <= k < 16:
                    if k == 0:
                        ps_o[(g, cb)] = ps.tile(
                            [128, QG], F32, tag="o", bufs=4, name=f"ps_o{g}_{cb}"
                        )
                    emit_pv(g, cb, 2 * k, start=(k == 0), stop=False)
                    emit_pv(g, cb, 2 * k + 1, start=False, stop=(k == 15))
                    if k == 15:
                        emit_out(g, cb)

    # ---- startup (stream 0): S/exp/Z for group 0, cb0/cb1 transposes,
    # and the head of group 0's PV chains
    for t in range(NM):
        if t % 2 == 0:
            emit_s(0, t)
            emit_s(0, t + 1)
        if t % 4 == 0 and t >= 4:
            emit_z_quad(0, t // 4 - 1)
        if 14 <= t < 22:
            emit_transpose_quad(0, t - 14)
        if t >= 24:
            emit_transpose_quad(1, t - 24)
        chain_emits(0, t)

    # ---- phases p = 0..7 (stream s = p+1 slots)
    for T in range(8 * 32):
        p, t = divmod(T, 32)
        if t == 0:
            emit_z_quad(p, NM // 4 - 1)
            emit_rz(p)
        if p == 0 and 2 <= t < 10:
            emit_transpose_quad(2, t - 2)
        if p == 0 and 10 <= t < 18:
            emit_transpose_quad(3, t - 10)
        chain_emits(p + 1, t)
        if p + 1 <= 7 and t < NM:
            if t % 2 == 0:
                emit_s(p + 1, t)
                emit_s(p + 1, t + 1)
            if t % 4 == 0 and t >= 4:
                emit_z_quad(p + 1, t // 4 - 1)


_prog_cache = {}


def _get_program(repeat: int = 1):
    if repeat not in _prog_cache:
        _prog_cache[repeat] = build_program(repeat)
    return _prog_cache[repeat]


def run(inputs, **spmd_kwargs):
    from concourse.bass_utils import run_bass_kernel_spmd

    Mk = np.ascontiguousarray(np.asarray(inputs["Mk"], dtype=np.float32))
    Qk = np.ascontiguousarray(np.asarray(inputs["Qk"], dtype=np.float32))
    mv = np.ascontiguousarray(np.asarray(inputs["mv"], dtype=np.float32))
    assert Mk.shape == (B, CK, H, W) and Qk.shape == (B, CK, H, W)
    assert mv.shape == (B, CV, H, W)

    in_maps = [
        {
            "Mk": Mk[b].reshape(CK, HW),
            "Qk": Qk[b].reshape(CK, HW),
            "mv": mv[b].reshape(CV, HW),
        }
        for b in range(B)
    ]
    nc = _get_program()
    res = run_bass_kernel_spmd(nc, in_maps, list(range(B)), **spmd_kwargs)
    out = np.stack([res.results[b]["out"] for b in range(B)])
    return out.reshape(B, CV, H, W).astype(np.float32), res


def kernel(**inputs) -> np.ndarray:
    out, _ = run(inputs)
    return out


# revision 9
# speedup vs baseline: 251.9812x; 1.0016x over previous
"""Bass/Trainium2 kernel for nn_AttentionMemory (scatter_memory), v2.

Reference computation (per batch b):
    S   = Mk^T @ Qk * (1/sqrt(CK))     # [HW, HW]
    P   = softmax(S, axis=memory)      # softmax over the m (row) axis
    out = mv @ P                       # [CV, HW]

Sharding: B=8 batches, one batch per NeuronCore (pure data parallel).

v2 schedule: fine-grained slot interleave. The S/exp/Z stream for group
g+1 is woven between the PV accumulation matmuls of group g so the
Activation engine (exp, 612 ns/tile) runs concurrently with PE instead of
gating a separate S phase. PV chains are staggered across "flat slots"
(chain (g,cb) occupies flat slots 32g+8cb .. +15 at 2 matmuls/slot) so
PSUM drains + out-multiplies spread out instead of bunching at group
boundaries. Z colsums ride in distinct PE column groups (tile_position)
and S matmuls in the two K=64 row halves, which run concurrently on HW.
rz broadcast moved to the idle GPSIMD engine (partition_broadcast).
"""

import numpy as np

import concourse.bass as bass
import concourse.mybir as mybir
import concourse.tile as tile
from concourse.masks import make_identity
from bass_rust import ScopedClock

B, CK, CV, H, W = 8, 64, 512, 64, 64
HW = H * W            # 4096
QG = 512              # q-group width (one PSUM bank of fp32)
NQ = HW // QG         # 8 q-groups
NM = HW // 128        # 32 m-chunks
NCB = CV // 128       # 4 c-blocks
SCALE = 1.0 / 8.0     # 1/sqrt(CK)

F32 = mybir.dt.float32
FP16 = mybir.dt.float16
BF16 = mybir.dt.bfloat16


class FixedTileContext(tile.TileContext):
    """Splits multi-wait sync_infos: this walrus accepts at most one sync
    wait per regular instruction (two on InstEventSemaphore). Extra waits
    move onto same-engine InstNoOp carriers inserted just before."""

    def _split_multi_waits(self, ordered):
        nc = self.nc
        for bb_name, insts in list(ordered.items()):
            new_insts = []
            changed = False
            for inst in insts:
                si = getattr(inst, "sync_info", None)
                waits = list(si.on_wait) if (si is not None and si.on_wait) else []
                limit = 2 if isinstance(inst, mybir.InstEventSemaphore) else 1
                if len(waits) > limit:
                    changed = True
                    for w in waits[limit:]:
                        new_insts.append(
                            mybir.InstNoOp(
                                name=nc.get_next_instruction_name(),
                                sync_info=mybir.SyncInfo(on_wait=[w], on_update=[]),
                                bass_nofuse=True,
                                engine=inst.engine,
                            )
                        )
                    inst.sync_info = mybir.SyncInfo(
                        on_wait=waits[:limit], on_update=list(si.on_update or [])
                    )
                new_insts.append(inst)
            if changed:
                ordered[bb_name] = new_insts

    def _lower_ordered_insts(self, ordered):
        self._split_multi_waits(ordered)
        return super()._lower_ordered_insts(ordered)

    def _drain_and_barrier(self, tick_clock, wait_clock):
        nc = self.nc
        drain_inst = nc.sync.drain()
        wait_clock.add_sem_waits(
            drain_inst.ins, ScopedClock({None: tick_clock.global_clock})
        )
        si = drain_inst.ins.sync_info
        waits = list(si.on_wait or []) if si is not None else []
        if len(waits) > 1:
            drain_inst.ins.sync_info = mybir.SyncInfo(
                on_wait=[waits[0]], on_update=list(si.on_update or [])
            )
            for w in waits[1:]:
                d2 = nc.sync.drain()
                d2.ins.sync_info = mybir.SyncInfo(on_wait=[w], on_update=[])
        nc.all_engine_barrier()
        assert self.sems is not None
        popped = nc._tile_sem_poison_stack.pop()
        assert popped is self._sem_poison
        nc.clear_and_free_semaphores(list(self.sems.allocated().values()))
        nc.all_engine_barrier()


def build_program(repeat: int = 1) -> bass.Bass:
    nc = bass.Bass()
    mk_d = nc.dram_tensor("Mk", [CK, HW], F32, kind="ExternalInput")
    qk_d = nc.dram_tensor("Qk", [CK, HW], F32, kind="ExternalInput")
    mv_d = nc.dram_tensor("mv", [CV, HW], F32, kind="ExternalInput")
    out_d = nc.dram_tensor("out", [CV, HW], F32, kind="ExternalOutput")

    with FixedTileContext(nc) as tc:
        with (
            tc.tile_pool(name="consts", bufs=1) as consts,
            tc.tile_pool(name="stage", bufs=2) as stage,
            tc.tile_pool(name="inp16", bufs=1) as inp16,
            tc.tile_pool(name="mvtp", bufs=1) as mvtp,
            tc.tile_pool(name="pp", bufs=2) as pp,
            tc.tile_pool(name="obp", bufs=2) as obp,
            tc.tile_pool(name="smallp", bufs=2) as smallp,
            tc.tile_pool(name="ps", bufs=2, space="PSUM") as ps,
        ):
            identity = consts.tile([128, 128], F32)
            make_identity(nc, identity[:])
            ident16 = consts.tile([128, 128], BF16)
            nc.vector.tensor_copy(ident16[:], identity[:])

            ones_h = consts.tile([128, 1], BF16)
            nc.gpsimd.memset(ones_h[:], 1.0)
            ones_r = consts.tile([1, 128], BF16)
            nc.gpsimd.memset(ones_r[:], 1.0)

            for _rep in range(repeat):
                emit_body(nc, tc, stage, inp16, mvtp, pp, obp, smallp, ps,
                          ident16, ones_h, ones_r, mk_d, qk_d, mv_d, out_d)
    return nc


def emit_body(nc, tc, stage, inp16, mvtp, pp, obp, smallp, ps,
              ident16, ones_h, ones_r, mk_d, qk_d, mv_d, out_d):
    # ---- HW warmup during the initial DMA wait (both invisible to the
    # cost-model sim, real on hardware):
    #  - dummy exp: pulls the ~1.3us activation-table load off the first
    #    real exp, which otherwise gates the S stream
    #  - dummy matmuls: keep the PE busy through the HAM activity window so
    #    the real S matmuls start at 2.4 GHz instead of the cold 1.2 GHz
    #    (PE-transposes don't count as HAM activity; matmuls do)
    warm_o = smallp.tile([128, 1], F32, tag="warm", bufs=1, name="warm_o")
    nc.scalar.activation(warm_o[:], ones_h[:],
                         mybir.ActivationFunctionType.Exp, scale=1.0)
    ps_warm = ps.tile([128, QG], F32, tag="s", name="ps_warm")
    for _ in range(60):
        nc.tensor.matmul(ps_warm[0:1, :128], ones_h[:], ident16[:],
                         start=True, stop=True)

    # ---- input load + cast to fp16, duplicated into both partition halves
    # (ch0 via double-DMA for latency; ch1-3 single-DMA + dup casts to save
    # DMA bandwidth for the mv loads). DMA order interleaves mv so every
    # consumer's data arrives just in time under aggregate-bandwidth limits.
    mk16 = inp16.tile([128, HW], FP16)
    qk16 = inp16.tile([128, HW], FP16)
    NCH = 4
    CW = HW // NCH
    mv_sb = []

    def emit_mv_dma(cb):
        t = stage.tile([128, HW], F32, tag="mv", name=f"mv_sb{cb}")
        nc.sync.dma_start(t[:], mv_d[cb * 128:(cb + 1) * 128, :])
        mv_sb.append(t)

    for ch in range(NCH):
        csl = slice(ch * CW, (ch + 1) * CW)
        for src_d, dst in ((mk_d, mk16), (qk_d, qk16)):
            if ch == 0:
                st = stage.tile([128, CW], F32, tag="mkqk")
                nc.sync.dma_start(st[:CK, :], src_d[:, csl])
                nc.sync.dma_start(st[CK:, :], src_d[:, csl])
                nc.vector.tensor_copy(dst[:, csl], st[:])
            else:
                st = stage.tile([64, CW], F32, tag="mkqk1")
                nc.sync.dma_start(st[:], src_d[:, csl])
                nc.vector.tensor_copy(dst[:CK, csl], st[:])
                nc.gpsimd.tensor_copy(dst[CK:, csl], st[:])
        if ch == 1:
            emit_mv_dma(0)
        elif ch == 3:
            for cb in range(1, NCB):
                emit_mv_dma(cb)

    # mvT[p, j, c] = mv[c, j*128+p], bf16 (PV stationary operand)
    mvT = mvtp.tile([128, NM, CV], BF16)

    P = [None] * NQ     # P[g]: [128, NM, QG] bf16, unnormalized exp
    ps_z = [None] * NQ  # Z colsum accumulators (4 col-group partials)
    rzb = [None] * NQ   # broadcast 1/Z rows
    ps_o = {}           # (g, cb) -> PV accumulation PSUM tile

    def emit_transpose_quad(cb, q):
        """Transpose m-chunks j=4q..4q+3 of mv c-block cb: 4 PE transposes
        into one PSUM tile, one DVE copy out (keeps the shared s-ring at
        ~2 allocs/slot)."""
        mq = stage.tile([128, QG], BF16, tag="mq", bufs=4, name="mq")
        nc.vector.tensor_copy(mq[:], mv_sb[cb][:, QG * q:QG * (q + 1)])
        ps_t = ps.tile([128, QG], BF16, tag="s", name="ps_t")
        for jj in range(4):
            nc.tensor.transpose(
                ps_t[:, jj * 128:(jj + 1) * 128],
                mq[:, jj * 128:(jj + 1) * 128], ident16[:]
            )
        nc.vector.tensor_copy(
            mvT[:, 4 * q:4 * q + 4, cb * 128:(cb + 1) * 128],
            ps_t.rearrange("p (j c) -> p j c", j=4),
        )

    def emit_s(g, j):
        """One S matmul + exp for (g, j). Allocates P[g]/ps_z[g] on j==0."""
        if j == 0:
            P[g] = pp.tile([128, NM, QG], BF16, tag="P", name=f"P{g}")
            ps_z[g] = ps.tile([128, QG], F32, tag="z", name=f"ps_z{g}")
        qsl = slice(g * QG, (g + 1) * QG)
        half = j % 2
        ksl = slice(half * CK, half * CK + CK)
        ps_sj = ps.tile([128, QG], F32, tag="s", name="ps_s")
        nc.tensor.matmul(
            ps_sj[:], mk16[ksl, j * 128:(j + 1) * 128], qk16[ksl, qsl],
            start=True, stop=True,
        )
        nc.scalar.activation(
            P[g][:, j, :], ps_sj[:],
            mybir.ActivationFunctionType.Exp, scale=SCALE,
        )

    def emit_z_quad(g, a):
        """Z colsum chunks j=4a..4a+3 for group g, emitted back-to-back so
        the 4 column-group chains run concurrently on the PE array."""
        for c in range(4):
            nc.tensor.matmul(
                ps_z[g][32 * c:32 * c + 1, :], ones_h[:], P[g][:, 4 * a + c, :],
                start=(a == 0), stop=(a == NM // 4 - 1),
                tile_position=(0, 32 * c),
            )

    def emit_rz(g):
        """Combine Z partials -> reciprocal -> broadcast (DVE + GPSIMD)."""
        za = smallp.tile([1, QG], F32, tag="zt", name="za")
        nc.vector.tensor_copy(za[:], ps_z[g][0:1, :])
        zb = smallp.tile([1, QG], F32, tag="zt", name="zb")
        nc.vector.tensor_tensor(
            out=zb[:], in0=za[:], in1=ps_z[g][32:33, :], op=mybir.AluOpType.add
        )
        zc = smallp.tile([1, QG], F32, tag="zt", name="zc")
        nc.vector.tensor_tensor(
            out=zc[:], in0=zb[:], in1=ps_z[g][64:65, :], op=mybir.AluOpType.add
        )
        zs = smallp.tile([1, QG], F32, tag="zt", name="zs")
        nc.vector.tensor_tensor(
            out=zs[:], in0=zc[:], in1=ps_z[g][96:97, :], op=mybir.AluOpType.add
        )
        rz = smallp.tile([1, QG], F32, tag="rz", name="rz")
        nc.vector.reciprocal(rz[:], zs[:])
        rz16 = smallp.tile([1, QG], BF16, tag="rz16", name="rz16")
        nc.vector.tensor_copy(rz16[:], rz[:])
        # broadcast along partitions: ones[1,128]^T @ rz16[1,QG] (bf16, 213ns)
        ps_rzb = ps.tile([128, QG], F32, tag="s", name="ps_rzb")
        nc.tensor.matmul(ps_rzb[:], ones_r[:], rz16[:], start=True, stop=True)
        rzb[g] = smallp.tile([128, QG], F32, tag="rzb", name=f"rzb{g}")
        nc.vector.tensor_copy(rzb[g][:], ps_rzb[:])

    def emit_pv(g, cb, j, start, stop):
        nc.tensor.matmul(
            ps_o[(g, cb)][:],
            mvT[:, j, cb * 128:(cb + 1) * 128],
            P[g][:, j, :],
            start=start, stop=stop,
        )

    def emit_out(g, cb):
        qsl = slice(g * QG, (g + 1) * QG)
        o_sb = obp.tile([128, QG], F32, tag="ob", name="o_sb")
        nc.vector.tensor_tensor(
            out=o_sb[:], in0=ps_o.pop((g, cb))[:], in1=rzb[g][:],
            op=mybir.AluOpType.mult,
        )
        nc.sync.dma_start(out_d[cb * 128:(cb + 1) * 128, qsl], o_sb[:])

    def emit_pv_half(g, cb, j, csl, start, stop, key):
        nc.tensor.matmul(
            ps_o[key][:],
            mvT[:, j, cb * 128:(cb + 1) * 128],
            P[g][:, j, csl],
            start=start, stop=stop,
        )

    def emit_out_half(g, cb, csl, key):
        qs = slice(g * QG + csl.start, g * QG + csl.stop)
        o_sb = obp.tile([128, QG // 2], F32, tag="obh", bufs=2, name="o_sbh")
        nc.vector.tensor_tensor(
            out=o_sb[:], in0=ps_o.pop(key)[:], in1=rzb[g][:, csl],
            op=mybir.AluOpType.mult,
        )
        nc.sync.dma_start(out_d[cb * 128:(cb + 1) * 128, qs], o_sb[:])

    def chain_emits(s, t):
        """PV chain work due at slot t of stream s. Chain (g, cb) occupies
        stream-g slots 18+8cb .. 31 and stream-(g+1) slots 0 .. 8cb+1.
        The very last chain (NQ-1, cb3) runs as two sequential q-halves so
        half A's out-mult + DMA overlap half B's matmuls (shorter tail)."""
        for cb in range(NCB):
            for g, k in ((s, t - 18 - 8 * cb), (s - 1, 32 + t - 18 - 8 * cb)):
                if not (0 <= g < NQ and 0 <= k < 16):
                    continue
                if g == NQ - 1 and cb == 3:
                    half = k // 8
                    csl = slice(half * (QG // 2), (half + 1) * (QG // 2))
                    key = (g, cb, half)
                    kk = k % 8
                    if kk == 0:
                        ps_o[key] = ps.tile(
                            [128, QG // 2], F32, tag="o", bufs=4,
                            name=f"ps_oh{half}"
                        )
                    for jj in range(4):
                        j = 4 * kk + jj
                        emit_pv_half(g, cb, j, csl,
                                     start=(j == 0), stop=(j == NM - 1), key=key)
                    if kk == 7:
                        emit_out_half(g, cb, csl, key)
                    continue
                if k == 0:
                    ps_o[(g, cb)] = ps.tile(
                        [128, QG], F32, tag="o", bufs=4, name=f"ps_o{g}_{cb}"
                    )
                emit_pv(g, cb, 2 * k, start=(k == 0), stop=False)
                emit_pv(g, cb, 2 * k + 1, start=False, stop=(k == 15))
                if k == 15:
                    emit_out(g, cb)

    # ---- startup (stream 0): S/exp/Z for group 0, cb0/cb1 transposes,
    # and the head of group 0's PV chains
    for t in range(NM):
        if t % 2 == 0:
            emit_s(0, t)
            emit_s(0, t + 1)
        if t % 4 == 0 and t >= 4:
            emit_z_quad(0, t // 4 - 1)
        if 14 <= t < 22:
            emit_transpose_quad(0, t - 14)
        if t >= 24:
            emit_transpose_quad(1, t - 24)
        chain_emits(0, t)

    # ---- phases p = 0..7 (stream s = p+1 slots)
    for T in range(8 * 32):
        p, t = divmod(T, 32)
        if t == 0:
            emit_z_quad(p, NM // 4 - 1)
            emit_rz(p)
        if p == 0 and 2 <= t < 10:
            emit_transpose_quad(2, t - 2)
        if p == 0 and 10 <= t < 18:
            emit_transpose_quad(3, t - 10)
        chain_emits(p + 1, t)
        if p + 1 <= 7 and t < NM:
            if t % 2 == 0:
                emit_s(p + 1, t)
                emit_s(p + 1, t + 1)
            if t % 4 == 0 and t >= 4:
                emit_z_quad(p + 1, t // 4 - 1)


_prog_cache = {}


def _get_program(repeat: int = 1):
    if repeat not in _prog_cache:
        _prog_cache[repeat] = build_program(repeat)
    return _prog_cache[repeat]


def run(inputs, **spmd_kwargs):
    from concourse.bass_utils import run_bass_kernel_spmd

    Mk = np.ascontiguousarray(np.asarray(inputs["Mk"], dtype=np.float32))
    Qk = np.ascontiguousarray(np.asarray(inputs["Qk"], dtype=np.float32))
    mv = np.ascontiguousarray(np.asarray(inputs["mv"], dtype=np.float32))
    assert Mk.shape == (B, CK, H, W) and Qk.shape == (B, CK, H, W)
    assert mv.shape == (B, CV, H, W)

    in_maps = [
        {
            "Mk": Mk[b].reshape(CK, HW),
            "Qk": Qk[b].reshape(CK, HW),
            "mv": mv[b].reshape(CV, HW),
        }
        for b in range(B)
    ]
    nc = _get_program()
    res = run_bass_kernel_spmd(nc, in_maps, list(range(B)), **spmd_kwargs)
    out = np.stack([res.results[b]["out"] for b in range(B)])
    return out.reshape(B, CV, H, W).astype(np.float32), res


def kernel(**inputs) -> np.ndarray:
    out, _ = run(inputs)
    return out


# revision 10
# speedup vs baseline: 253.6313x; 1.0065x over previous
"""Bass/Trainium2 kernel for nn_AttentionMemory (scatter_memory), v2.

Reference computation (per batch b):
    S   = Mk^T @ Qk * (1/sqrt(CK))     # [HW, HW]
    P   = softmax(S, axis=memory)      # softmax over the m (row) axis
    out = mv @ P                       # [CV, HW]

Sharding: B=8 batches, one batch per NeuronCore (pure data parallel).

v2 schedule: fine-grained slot interleave. The S/exp/Z stream for group
g+1 is woven between the PV accumulation matmuls of group g so the
Activation engine (exp, 612 ns/tile) runs concurrently with PE instead of
gating a separate S phase. PV chains are staggered across "flat slots"
(chain (g,cb) occupies flat slots 32g+8cb .. +15 at 2 matmuls/slot) so
PSUM drains + out-multiplies spread out instead of bunching at group
boundaries. Z colsums ride in distinct PE column groups (tile_position)
and S matmuls in the two K=64 row halves, which run concurrently on HW.
rz broadcast moved to the idle GPSIMD engine (partition_broadcast).
"""

import numpy as np

import concourse.bass as bass
import concourse.mybir as mybir
import concourse.tile as tile
from concourse.masks import make_identity
from bass_rust import ScopedClock

B, CK, CV, H, W = 8, 64, 512, 64, 64
HW = H * W            # 4096
QG = 512              # q-group width (one PSUM bank of fp32)
NQ = HW // QG         # 8 q-groups
NM = HW // 128        # 32 m-chunks
NCB = CV // 128       # 4 c-blocks
SCALE = 1.0 / 8.0     # 1/sqrt(CK)

F32 = mybir.dt.float32
FP16 = mybir.dt.float16
BF16 = mybir.dt.bfloat16


class FixedTileContext(tile.TileContext):
    """Splits multi-wait sync_infos: this walrus accepts at most one sync
    wait per regular instruction (two on InstEventSemaphore). Extra waits
    move onto same-engine InstNoOp carriers inserted just before."""

    def _split_multi_waits(self, ordered):
        nc = self.nc
        for bb_name, insts in list(ordered.items()):
            new_insts = []
            changed = False
            for inst in insts:
                si = getattr(inst, "sync_info", None)
                waits = list(si.on_wait) if (si is not None and si.on_wait) else []
                limit = 2 if isinstance(inst, mybir.InstEventSemaphore) else 1
                if len(waits) > limit:
                    changed = True
                    for w in waits[limit:]:
                        new_insts.append(
                            mybir.InstNoOp(
                                name=nc.get_next_instruction_name(),
                                sync_info=mybir.SyncInfo(on_wait=[w], on_update=[]),
                                bass_nofuse=True,
                                engine=inst.engine,
                            )
                        )
                    inst.sync_info = mybir.SyncInfo(
                        on_wait=waits[:limit], on_update=list(si.on_update or [])
                    )
                new_insts.append(inst)
            if changed:
                ordered[bb_name] = new_insts

    def _lower_ordered_insts(self, ordered):
        self._split_multi_waits(ordered)
        return super()._lower_ordered_insts(ordered)

    def _drain_and_barrier(self, tick_clock, wait_clock):
        nc = self.nc
        drain_inst = nc.sync.drain()
        wait_clock.add_sem_waits(
            drain_inst.ins, ScopedClock({None: tick_clock.global_clock})
        )
        si = drain_inst.ins.sync_info
        waits = list(si.on_wait or []) if si is not None else []
        if len(waits) > 1:
            drain_inst.ins.sync_info = mybir.SyncInfo(
                on_wait=[waits[0]], on_update=list(si.on_update or [])
            )
            for w in waits[1:]:
                d2 = nc.sync.drain()
                d2.ins.sync_info = mybir.SyncInfo(on_wait=[w], on_update=[])
        nc.all_engine_barrier()
        assert self.sems is not None
        popped = nc._tile_sem_poison_stack.pop()
        assert popped is self._sem_poison
        nc.clear_and_free_semaphores(list(self.sems.allocated().values()))
        nc.all_engine_barrier()


def build_program(repeat: int = 1) -> bass.Bass:
    nc = bass.Bass()
    mk_d = nc.dram_tensor("Mk", [CK, HW], F32, kind="ExternalInput")
    qk_d = nc.dram_tensor("Qk", [CK, HW], F32, kind="ExternalInput")
    mv_d = nc.dram_tensor("mv", [CV, HW], F32, kind="ExternalInput")
    out_d = nc.dram_tensor("out", [CV, HW], F32, kind="ExternalOutput")

    with FixedTileContext(nc) as tc:
        with (
            tc.tile_pool(name="consts", bufs=1) as consts,
            tc.tile_pool(name="stage", bufs=2) as stage,
            tc.tile_pool(name="inp16", bufs=1) as inp16,
            tc.tile_pool(name="mvtp", bufs=1) as mvtp,
            tc.tile_pool(name="pp", bufs=2) as pp,
            tc.tile_pool(name="obp", bufs=2) as obp,
            tc.tile_pool(name="smallp", bufs=2) as smallp,
            tc.tile_pool(name="ps", bufs=2, space="PSUM") as ps,
        ):
            identity = consts.tile([128, 128], F32)
            make_identity(nc, identity[:])
            ident16 = consts.tile([128, 128], BF16)
            nc.vector.tensor_copy(ident16[:], identity[:])

            ones_h = consts.tile([128, 1], BF16)
            nc.gpsimd.memset(ones_h[:], 1.0)
            ones_r = consts.tile([1, 128], BF16)
            nc.gpsimd.memset(ones_r[:], 1.0)

            for _rep in range(repeat):
                emit_body(nc, tc, stage, inp16, mvtp, pp, obp, smallp, ps,
                          ident16, ones_h, ones_r, mk_d, qk_d, mv_d, out_d)
    return nc


def emit_body(nc, tc, stage, inp16, mvtp, pp, obp, smallp, ps,
              ident16, ones_h, ones_r, mk_d, qk_d, mv_d, out_d):
    # ---- HW warmup during the initial DMA wait (both invisible to the
    # cost-model sim, real on hardware):
    #  - dummy exp: pulls the ~1.3us activation-table load off the first
    #    real exp, which otherwise gates the S stream
    #  - dummy matmuls: keep the PE busy through the HAM activity window so
    #    the real S matmuls start at 2.4 GHz instead of the cold 1.2 GHz
    #    (PE-transposes don't count as HAM activity; matmuls do)
    warm_o = smallp.tile([128, 1], F32, tag="warm", bufs=1, name="warm_o")
    nc.scalar.activation(warm_o[:], ones_h[:],
                         mybir.ActivationFunctionType.Exp, scale=1.0)
    ps_warm = ps.tile([128, QG], F32, tag="s", name="ps_warm")
    for _ in range(60):
        nc.tensor.matmul(ps_warm[0:1, :128], ones_h[:], ident16[:],
                         start=True, stop=True)

    # ---- input load + cast to fp16, duplicated into both partition halves
    # (ch0 via double-DMA for latency; ch1-3 single-DMA + dup casts to save
    # DMA bandwidth for the mv loads). DMA order interleaves mv so every
    # consumer's data arrives just in time under aggregate-bandwidth limits.
    mk16 = inp16.tile([128, HW], FP16)
    qk16 = inp16.tile([128, HW], FP16)
    NCH = 4
    CW = HW // NCH
    mv_sb = []

    def emit_mv_dma(cb):
        t = stage.tile([128, HW], F32, tag="mv", name=f"mv_sb{cb}")
        nc.sync.dma_start(t[:], mv_d[cb * 128:(cb + 1) * 128, :])
        mv_sb.append(t)

    for ch in range(NCH):
        csl = slice(ch * CW, (ch + 1) * CW)
        for src_d, dst in ((mk_d, mk16), (qk_d, qk16)):
            if ch == 0:
                st = stage.tile([128, CW], F32, tag="mkqk")
                nc.sync.dma_start(st[:CK, :], src_d[:, csl])
                nc.sync.dma_start(st[CK:, :], src_d[:, csl])
                nc.vector.tensor_copy(dst[:, csl], st[:])
            else:
                st = stage.tile([64, CW], F32, tag="mkqk1")
                nc.sync.dma_start(st[:], src_d[:, csl])
                nc.vector.tensor_copy(dst[:CK, csl], st[:])
                nc.gpsimd.tensor_copy(dst[CK:, csl], st[:])
        if ch == 1:
            emit_mv_dma(0)
        elif ch == 3:
            for cb in range(1, NCB):
                emit_mv_dma(cb)

    # mvT[p, j, c] = mv[c, j*128+p], bf16 (PV stationary operand)
    mvT = mvtp.tile([128, NM, CV], BF16)

    P = [None] * NQ     # P[g]: [128, NM, QG] bf16, unnormalized exp
    ps_z = [None] * NQ  # Z colsum accumulators (4 col-group partials)
    rzb = [None] * NQ   # broadcast 1/Z rows
    ps_o = {}           # (g, cb) -> PV accumulation PSUM tile

    def emit_transpose_quad(cb, q):
        """Transpose m-chunks j=4q..4q+3 of mv c-block cb: 4 PE transposes
        into one PSUM tile, one DVE copy out (keeps the shared s-ring at
        ~2 allocs/slot)."""
        mq = stage.tile([128, QG], BF16, tag="mq", bufs=4, name="mq")
        nc.vector.tensor_copy(mq[:], mv_sb[cb][:, QG * q:QG * (q + 1)])
        ps_t = ps.tile([128, QG], BF16, tag="o", bufs=4, name="ps_t")
        for jj in range(4):
            nc.tensor.transpose(
                ps_t[:, jj * 128:(jj + 1) * 128],
                mq[:, jj * 128:(jj + 1) * 128], ident16[:]
            )
        nc.vector.tensor_copy(
            mvT[:, 4 * q:4 * q + 4, cb * 128:(cb + 1) * 128],
            ps_t.rearrange("p (j c) -> p j c", j=4),
        )

    def emit_s(g, j):
        """One S matmul + exp for (g, j). Allocates P[g]/ps_z[g] on j==0."""
        if j == 0:
            P[g] = pp.tile([128, NM, QG], BF16, tag="P", name=f"P{g}")
            ps_z[g] = ps.tile([128, QG], F32, tag="z", name=f"ps_z{g}")
        qsl = slice(g * QG, (g + 1) * QG)
        half = j % 2
        ksl = slice(half * CK, half * CK + CK)
        ps_sj = ps.tile([128, QG], F32, tag="s", name="ps_s")
        nc.tensor.matmul(
            ps_sj[:], mk16[ksl, j * 128:(j + 1) * 128], qk16[ksl, qsl],
            start=True, stop=True,
        )
        nc.scalar.activation(
            P[g][:, j, :], ps_sj[:],
            mybir.ActivationFunctionType.Exp, scale=SCALE,
        )

    def emit_z_quad(g, a):
        """Z colsum chunks j=4a..4a+3 for group g, emitted back-to-back so
        the 4 column-group chains run concurrently on the PE array."""
        for c in range(4):
            nc.tensor.matmul(
                ps_z[g][32 * c:32 * c + 1, :], ones_h[:], P[g][:, 4 * a + c, :],
                start=(a == 0), stop=(a == NM // 4 - 1),
                tile_position=(0, 32 * c),
            )

    def emit_rz(g):
        """Combine Z partials -> reciprocal -> broadcast (DVE + GPSIMD)."""
        za = smallp.tile([1, QG], F32, tag="zt", name="za")
        nc.vector.tensor_copy(za[:], ps_z[g][0:1, :])
        zb = smallp.tile([1, QG], F32, tag="zt", name="zb")
        nc.vector.tensor_tensor(
            out=zb[:], in0=za[:], in1=ps_z[g][32:33, :], op=mybir.AluOpType.add
        )
        zc = smallp.tile([1, QG], F32, tag="zt", name="zc")
        nc.vector.tensor_tensor(
            out=zc[:], in0=zb[:], in1=ps_z[g][64:65, :], op=mybir.AluOpType.add
        )
        zs = smallp.tile([1, QG], F32, tag="zt", name="zs")
        nc.vector.tensor_tensor(
            out=zs[:], in0=zc[:], in1=ps_z[g][96:97, :], op=mybir.AluOpType.add
        )
        rz = smallp.tile([1, QG], F32, tag="rz", name="rz")
        nc.vector.reciprocal(rz[:], zs[:])
        rz16 = smallp.tile([1, QG], BF16, tag="rz16", name="rz16")
        nc.vector.tensor_copy(rz16[:], rz[:])
        # broadcast along partitions: ones[1,128]^T @ rz16[1,QG] (bf16, 213ns)
        ps_rzb = ps.tile([128, QG], F32, tag="s", name="ps_rzb")
        nc.tensor.matmul(ps_rzb[:], ones_r[:], rz16[:], start=True, stop=True)
        rzb[g] = smallp.tile([128, QG], F32, tag="rzb", name=f"rzb{g}")
        nc.vector.tensor_copy(rzb[g][:], ps_rzb[:])

    def emit_pv(g, cb, j, start, stop):
        nc.tensor.matmul(
            ps_o[(g, cb)][:],
            mvT[:, j, cb * 128:(cb + 1) * 128],
            P[g][:, j, :],
            start=start, stop=stop,
        )

    def emit_out(g, cb):
        qsl = slice(g * QG, (g + 1) * QG)
        o_sb = obp.tile([128, QG], F32, tag="ob", name="o_sb")
        nc.vector.tensor_tensor(
            out=o_sb[:], in0=ps_o.pop((g, cb))[:], in1=rzb[g][:],
            op=mybir.AluOpType.mult,
        )
        nc.sync.dma_start(out_d[cb * 128:(cb + 1) * 128, qsl], o_sb[:])

    def emit_pv_half(g, cb, j, csl, start, stop, key):
        nc.tensor.matmul(
            ps_o[key][:],
            mvT[:, j, cb * 128:(cb + 1) * 128],
            P[g][:, j, csl],
            start=start, stop=stop,
        )

    def emit_out_half(g, cb, csl, key):
        qs = slice(g * QG + csl.start, g * QG + csl.stop)
        o_sb = obp.tile([128, QG // 2], F32, tag="obh", bufs=2, name="o_sbh")
        nc.vector.tensor_tensor(
            out=o_sb[:], in0=ps_o.pop(key)[:], in1=rzb[g][:, csl],
            op=mybir.AluOpType.mult,
        )
        nc.sync.dma_start(out_d[cb * 128:(cb + 1) * 128, qs], o_sb[:])

    def chain_emits(s, t):
        """PV chain work due at slot t of stream s. Chain (g, cb) occupies
        stream-g slots 18+8cb .. 31 and stream-(g+1) slots 0 .. 8cb+1.
        The very last chain (NQ-1, cb3) runs as two sequential q-halves so
        half A's out-mult + DMA overlap half B's matmuls (shorter tail)."""
        for cb in range(NCB):
            for g, k in ((s, t - 18 - 8 * cb), (s - 1, 32 + t - 18 - 8 * cb)):
                if not (0 <= g < NQ and 0 <= k < 16):
                    continue
                if g == NQ - 1 and cb == 3:
                    half = k // 8
                    csl = slice(half * (QG // 2), (half + 1) * (QG // 2))
                    key = (g, cb, half)
                    kk = k % 8
                    if kk == 0:
                        ps_o[key] = ps.tile(
                            [128, QG // 2], F32, tag="o", bufs=4,
                            name=f"ps_oh{half}"
                        )
                    for jj in range(4):
                        j = 4 * kk + jj
                        emit_pv_half(g, cb, j, csl,
                                     start=(j == 0), stop=(j == NM - 1), key=key)
                    if kk == 7:
                        emit_out_half(g, cb, csl, key)
                    continue
                if k == 0:
                    ps_o[(g, cb)] = ps.tile(
                        [128, QG], F32, tag="o", bufs=4, name=f"ps_o{g}_{cb}"
                    )
                emit_pv(g, cb, 2 * k, start=(k == 0), stop=False)
                emit_pv(g, cb, 2 * k + 1, start=False, stop=(k == 15))
                if k == 15:
                    emit_out(g, cb)

    # ---- startup (stream 0): S/exp/Z for group 0, cb0/cb1 transposes,
    # and the head of group 0's PV chains
    for t in range(NM):
        if t % 2 == 0:
            emit_s(0, t)
            emit_s(0, t + 1)
        if t % 4 == 0 and t >= 4:
            emit_z_quad(0, t // 4 - 1)
        if 14 <= t < 22:
            emit_transpose_quad(0, t - 14)
        if t >= 24:
            emit_transpose_quad(1, t - 24)
        chain_emits(0, t)

    # ---- phases p = 0..7 (stream s = p+1 slots)
    for T in range(8 * 32):
        p, t = divmod(T, 32)
        if t == 0:
            emit_z_quad(p, NM // 4 - 1)
            emit_rz(p)
        if p == 0 and 2 <= t < 10:
            emit_transpose_quad(2, t - 2)
        if p == 0 and 10 <= t < 18:
            emit_transpose_quad(3, t - 10)
        chain_emits(p + 1, t)
        if p + 1 <= 7 and t < NM:
            if t % 2 == 0:
                emit_s(p + 1, t)
                emit_s(p + 1, t + 1)
            if t % 4 == 0 and t >= 4:
                emit_z_quad(p + 1, t // 4 - 1)


_prog_cache = {}


def _get_program(repeat: int = 1):
    if repeat not in _prog_cache:
        _prog_cache[repeat] = build_program(repeat)
    return _prog_cache[repeat]


def run(inputs, **spmd_kwargs):
    from concourse.bass_utils import run_bass_kernel_spmd

    Mk = np.ascontiguousarray(np.asarray(inputs["Mk"], dtype=np.float32))
    Qk = np.ascontiguousarray(np.asarray(inputs["Qk"], dtype=np.float32))
    mv = np.ascontiguousarray(np.asarray(inputs["mv"], dtype=np.float32))
    assert Mk.shape == (B, CK, H, W) and Qk.shape == (B, CK, H, W)
    assert mv.shape == (B, CV, H, W)

    in_maps = [
        {
            "Mk": Mk[b].reshape(CK, HW),
            "Qk": Qk[b].reshape(CK, HW),
            "mv": mv[b].reshape(CV, HW),
        }
        for b in range(B)
    ]
    nc = _get_program()
    res = run_bass_kernel_spmd(nc, in_maps, list(range(B)), **spmd_kwargs)
    out = np.stack([res.results[b]["out"] for b in range(B)])
    return out.reshape(B, CV, H, W).astype(np.float32), res


def kernel(**inputs) -> np.ndarray:
    out, _ = run(inputs)
    return out


# revision 11
# speedup vs baseline: 254.4482x; 1.0032x over previous
"""Bass/Trainium2 kernel for nn_AttentionMemory (scatter_memory), v2.

Reference computation (per batch b):
    S   = Mk^T @ Qk * (1/sqrt(CK))     # [HW, HW]
    P   = softmax(S, axis=memory)      # softmax over the m (row) axis
    out = mv @ P                       # [CV, HW]

Sharding: B=8 batches, one batch per NeuronCore (pure data parallel).

v2 schedule: fine-grained slot interleave. The S/exp/Z stream for group
g+1 is woven between the PV accumulation matmuls of group g so the
Activation engine (exp, 612 ns/tile) runs concurrently with PE instead of
gating a separate S phase. PV chains are staggered across "flat slots"
(chain (g,cb) occupies flat slots 32g+8cb .. +15 at 2 matmuls/slot) so
PSUM drains + out-multiplies spread out instead of bunching at group
boundaries. Z colsums ride in distinct PE column groups (tile_position)
and S matmuls in the two K=64 row halves, which run concurrently on HW.
rz broadcast moved to the idle GPSIMD engine (partition_broadcast).
"""

import numpy as np

import concourse.bass as bass
import concourse.mybir as mybir
import concourse.tile as tile
from concourse.masks import make_identity
from bass_rust import ScopedClock

B, CK, CV, H, W = 8, 64, 512, 64, 64
HW = H * W            # 4096
QG = 512              # q-group width (one PSUM bank of fp32)
NQ = HW // QG         # 8 q-groups
NM = HW // 128        # 32 m-chunks
NCB = CV // 128       # 4 c-blocks
SCALE = 1.0 / 8.0     # 1/sqrt(CK)

F32 = mybir.dt.float32
FP16 = mybir.dt.float16
BF16 = mybir.dt.bfloat16


class FixedTileContext(tile.TileContext):
    """Splits multi-wait sync_infos: this walrus accepts at most one sync
    wait per regular instruction (two on InstEventSemaphore). Extra waits
    move onto same-engine InstNoOp carriers inserted just before."""

    def _split_multi_waits(self, ordered):
        nc = self.nc
        for bb_name, insts in list(ordered.items()):
            new_insts = []
            changed = False
            for inst in insts:
                si = getattr(inst, "sync_info", None)
                waits = list(si.on_wait) if (si is not None and si.on_wait) else []
                limit = 2 if isinstance(inst, mybir.InstEventSemaphore) else 1
                if len(waits) > limit:
                    changed = True
                    for w in waits[limit:]:
                        new_insts.append(
                            mybir.InstNoOp(
                                name=nc.get_next_instruction_name(),
                                sync_info=mybir.SyncInfo(on_wait=[w], on_update=[]),
                                bass_nofuse=True,
                                engine=inst.engine,
                            )
                        )
                    inst.sync_info = mybir.SyncInfo(
                        on_wait=waits[:limit], on_update=list(si.on_update or [])
                    )
                new_insts.append(inst)
            if changed:
                ordered[bb_name] = new_insts

    def _lower_ordered_insts(self, ordered):
        self._split_multi_waits(ordered)
        return super()._lower_ordered_insts(ordered)

    def _drain_and_barrier(self, tick_clock, wait_clock):
        nc = self.nc
        drain_inst = nc.sync.drain()
        wait_clock.add_sem_waits(
            drain_inst.ins, ScopedClock({None: tick_clock.global_clock})
        )
        si = drain_inst.ins.sync_info
        waits = list(si.on_wait or []) if si is not None else []
        if len(waits) > 1:
            drain_inst.ins.sync_info = mybir.SyncInfo(
                on_wait=[waits[0]], on_update=list(si.on_update or [])
            )
            for w in waits[1:]:
                d2 = nc.sync.drain()
                d2.ins.sync_info = mybir.SyncInfo(on_wait=[w], on_update=[])
        nc.all_engine_barrier()
        assert self.sems is not None
        popped = nc._tile_sem_poison_stack.pop()
        assert popped is self._sem_poison
        nc.clear_and_free_semaphores(list(self.sems.allocated().values()))
        nc.all_engine_barrier()


def build_program(repeat: int = 1) -> bass.Bass:
    nc = bass.Bass()
    mk_d = nc.dram_tensor("Mk", [CK, HW], F32, kind="ExternalInput")
    qk_d = nc.dram_tensor("Qk", [CK, HW], F32, kind="ExternalInput")
    mv_d = nc.dram_tensor("mv", [CV, HW], F32, kind="ExternalInput")
    out_d = nc.dram_tensor("out", [CV, HW], F32, kind="ExternalOutput")

    with FixedTileContext(nc) as tc:
        with (
            tc.tile_pool(name="consts", bufs=1) as consts,
            tc.tile_pool(name="stage", bufs=2) as stage,
            tc.tile_pool(name="inp16", bufs=1) as inp16,
            tc.tile_pool(name="mvtp", bufs=1) as mvtp,
            tc.tile_pool(name="pp", bufs=2) as pp,
            tc.tile_pool(name="obp", bufs=2) as obp,
            tc.tile_pool(name="smallp", bufs=2) as smallp,
            tc.tile_pool(name="ps", bufs=2, space="PSUM") as ps,
        ):
            identity = consts.tile([128, 128], F32)
            make_identity(nc, identity[:])
            ident16 = consts.tile([128, 128], BF16)
            nc.vector.tensor_copy(ident16[:], identity[:])

            ones_h = consts.tile([128, 1], BF16)
            nc.gpsimd.memset(ones_h[:], 1.0)
            ones_r = consts.tile([1, 128], BF16)
            nc.gpsimd.memset(ones_r[:], 1.0)

            for _rep in range(repeat):
                emit_body(nc, tc, stage, inp16, mvtp, pp, obp, smallp, ps,
                          ident16, ones_h, ones_r, mk_d, qk_d, mv_d, out_d)
    return nc


def emit_body(nc, tc, stage, inp16, mvtp, pp, obp, smallp, ps,
              ident16, ones_h, ones_r, mk_d, qk_d, mv_d, out_d):
    # ---- HW warmup during the initial DMA wait (both invisible to the
    # cost-model sim, real on hardware):
    #  - dummy exp: pulls the ~1.3us activation-table load off the first
    #    real exp, which otherwise gates the S stream
    #  - dummy matmuls: keep the PE busy through the HAM activity window so
    #    the real S matmuls start at 2.4 GHz instead of the cold 1.2 GHz
    #    (PE-transposes don't count as HAM activity; matmuls do)
    warm_o = smallp.tile([128, 1], F32, tag="warm", bufs=1, name="warm_o")
    nc.scalar.activation(warm_o[:], ones_h[:],
                         mybir.ActivationFunctionType.Exp, scale=1.0)
    ps_warm = ps.tile([128, QG], F32, tag="s", name="ps_warm")
    for _ in range(16):
        nc.tensor.matmul(ps_warm[0:1, 0:1], ones_h[:], ones_h[:],
                         start=True, stop=True)
    for _ in range(60):
        nc.tensor.matmul(ps_warm[0:1, :128], ones_h[:], ident16[:],
                         start=True, stop=True)

    # ---- input load + cast to fp16, duplicated into both partition halves
    # (ch0 via double-DMA for latency; ch1-3 single-DMA + dup casts to save
    # DMA bandwidth for the mv loads). DMA order interleaves mv so every
    # consumer's data arrives just in time under aggregate-bandwidth limits.
    mk16 = inp16.tile([128, HW], FP16)
    qk16 = inp16.tile([128, HW], FP16)
    NCH = 4
    CW = HW // NCH
    mv_sb = []

    def emit_mv_dma(cb):
        t = stage.tile([128, HW], F32, tag="mv", name=f"mv_sb{cb}")
        nc.sync.dma_start(t[:], mv_d[cb * 128:(cb + 1) * 128, :])
        mv_sb.append(t)

    for ch in range(NCH):
        csl = slice(ch * CW, (ch + 1) * CW)
        for src_d, dst in ((mk_d, mk16), (qk_d, qk16)):
            if ch == 0:
                st = stage.tile([128, CW], F32, tag="mkqk")
                nc.sync.dma_start(st[:CK, :], src_d[:, csl])
                nc.sync.dma_start(st[CK:, :], src_d[:, csl])
                nc.vector.tensor_copy(dst[:, csl], st[:])
            else:
                st = stage.tile([64, CW], F32, tag="mkqk1")
                nc.sync.dma_start(st[:], src_d[:, csl])
                nc.vector.tensor_copy(dst[:CK, csl], st[:])
                nc.gpsimd.tensor_copy(dst[CK:, csl], st[:])
        if ch == 0:
            # first half of mv0 right after ch0: cb0's first 4 transpose
            # quads (m-chunks 0..15) can start ~4us earlier
            t0 = stage.tile([128, HW], F32, tag="mv", name="mv_sb0")
            nc.sync.dma_start(t0[:, :HW // 2], mv_d[0:128, :HW // 2])
            mv_sb.append(t0)
        elif ch == 2:
            nc.sync.dma_start(mv_sb[0][:, HW // 2:], mv_d[0:128, HW // 2:])
        elif ch == 3:
            for cb in range(1, NCB):
                emit_mv_dma(cb)

    # mvT[p, j, c] = mv[c, j*128+p], bf16 (PV stationary operand)
    mvT = mvtp.tile([128, NM, CV], BF16)

    P = [None] * NQ     # P[g]: [128, NM, QG] bf16, unnormalized exp
    ps_z = [None] * NQ  # Z colsum accumulators (4 col-group partials)
    rzb = [None] * NQ   # broadcast 1/Z rows
    ps_o = {}           # (g, cb) -> PV accumulation PSUM tile

    def emit_transpose_quad(cb, q):
        """Transpose m-chunks j=4q..4q+3 of mv c-block cb: 4 PE transposes
        into one PSUM tile, one DVE copy out (keeps the shared s-ring at
        ~2 allocs/slot)."""
        mq = stage.tile([128, QG], BF16, tag="mq", bufs=4, name="mq")
        nc.vector.tensor_copy(mq[:], mv_sb[cb][:, QG * q:QG * (q + 1)])
        ps_t = ps.tile([128, QG], BF16, tag="o", bufs=4, name="ps_t")
        for jj in range(4):
            nc.tensor.transpose(
                ps_t[:, jj * 128:(jj + 1) * 128],
                mq[:, jj * 128:(jj + 1) * 128], ident16[:]
            )
        nc.vector.tensor_copy(
            mvT[:, 4 * q:4 * q + 4, cb * 128:(cb + 1) * 128],
            ps_t.rearrange("p (j c) -> p j c", j=4),
        )

    def emit_s(g, j):
        """One S matmul + exp for (g, j). Allocates P[g]/ps_z[g] on j==0."""
        if j == 0:
            P[g] = pp.tile([128, NM, QG], BF16, tag="P", name=f"P{g}")
            ps_z[g] = ps.tile([128, QG], F32, tag="z", name=f"ps_z{g}")
        qsl = slice(g * QG, (g + 1) * QG)
        half = j % 2
        ksl = slice(half * CK, half * CK + CK)
        ps_sj = ps.tile([128, QG], F32, tag="s", name="ps_s")
        nc.tensor.matmul(
            ps_sj[:], mk16[ksl, j * 128:(j + 1) * 128], qk16[ksl, qsl],
            start=True, stop=True,
        )
        nc.scalar.activation(
            P[g][:, j, :], ps_sj[:],
            mybir.ActivationFunctionType.Exp, scale=SCALE,
        )

    def emit_z_quad(g, a):
        """Z colsum chunks j=4a..4a+3 for group g, emitted back-to-back so
        the 4 column-group chains run concurrently on the PE array."""
        for c in range(4):
            nc.tensor.matmul(
                ps_z[g][32 * c:32 * c + 1, :], ones_h[:], P[g][:, 4 * a + c, :],
                start=(a == 0), stop=(a == NM // 4 - 1),
                tile_position=(0, 32 * c),
            )

    def emit_rz(g):
        """Combine Z partials -> reciprocal -> broadcast (DVE + GPSIMD)."""
        za = smallp.tile([1, QG], F32, tag="zt", name="za")
        nc.vector.tensor_copy(za[:], ps_z[g][0:1, :])
        zb = smallp.tile([1, QG], F32, tag="zt", name="zb")
        nc.vector.tensor_tensor(
            out=zb[:], in0=za[:], in1=ps_z[g][32:33, :], op=mybir.AluOpType.add
        )
        zc = smallp.tile([1, QG], F32, tag="zt", name="zc")
        nc.vector.tensor_tensor(
            out=zc[:], in0=zb[:], in1=ps_z[g][64:65, :], op=mybir.AluOpType.add
        )
        zs = smallp.tile([1, QG], F32, tag="zt", name="zs")
        nc.vector.tensor_tensor(
            out=zs[:], in0=zc[:], in1=ps_z[g][96:97, :], op=mybir.AluOpType.add
        )
        rz = smallp.tile([1, QG], F32, tag="rz", name="rz")
        nc.vector.reciprocal(rz[:], zs[:])
        rz16 = smallp.tile([1, QG], BF16, tag="rz16", name="rz16")
        nc.vector.tensor_copy(rz16[:], rz[:])
        # broadcast along partitions: ones[1,128]^T @ rz16[1,QG] (bf16, 213ns)
        ps_rzb = ps.tile([128, QG], F32, tag="s", name="ps_rzb")
        nc.tensor.matmul(ps_rzb[:], ones_r[:], rz16[:], start=True, stop=True)
        rzb[g] = smallp.tile([128, QG], F32, tag="rzb", name=f"rzb{g}")
        nc.vector.tensor_copy(rzb[g][:], ps_rzb[:])

    def emit_pv(g, cb, j, start, stop):
        nc.tensor.matmul(
            ps_o[(g, cb)][:],
            mvT[:, j, cb * 128:(cb + 1) * 128],
            P[g][:, j, :],
            start=start, stop=stop,
        )

    def emit_out(g, cb):
        qsl = slice(g * QG, (g + 1) * QG)
        o_sb = obp.tile([128, QG], F32, tag="ob", name="o_sb")
        nc.vector.tensor_tensor(
            out=o_sb[:], in0=ps_o.pop((g, cb))[:], in1=rzb[g][:],
            op=mybir.AluOpType.mult,
        )
        nc.sync.dma_start(out_d[cb * 128:(cb + 1) * 128, qsl], o_sb[:])

    def emit_pv_half(g, cb, j, csl, start, stop, key):
        nc.tensor.matmul(
            ps_o[key][:],
            mvT[:, j, cb * 128:(cb + 1) * 128],
            P[g][:, j, csl],
            start=start, stop=stop,
        )

    def emit_out_half(g, cb, csl, key):
        qs = slice(g * QG + csl.start, g * QG + csl.stop)
        o_sb = obp.tile([128, QG // 2], F32, tag="obh", bufs=2, name="o_sbh")
        nc.vector.tensor_tensor(
            out=o_sb[:], in0=ps_o.pop(key)[:], in1=rzb[g][:, csl],
            op=mybir.AluOpType.mult,
        )
        nc.sync.dma_start(out_d[cb * 128:(cb + 1) * 128, qs], o_sb[:])

    def chain_emits(s, t):
        """PV chain work due at slot t of stream s. Chain (g, cb) occupies
        stream-g slots 18+8cb .. 31 and stream-(g+1) slots 0 .. 8cb+1.
        The very last chain (NQ-1, cb3) runs as two sequential q-halves so
        half A's out-mult + DMA overlap half B's matmuls (shorter tail)."""
        for cb in range(NCB):
            for g, k in ((s, t - 18 - 8 * cb), (s - 1, 32 + t - 18 - 8 * cb)):
                if not (0 <= g < NQ and 0 <= k < 16):
                    continue
                if g == NQ - 1 and cb == 3:
                    half = k // 8
                    csl = slice(half * (QG // 2), (half + 1) * (QG // 2))
                    key = (g, cb, half)
                    kk = k % 8
                    if kk == 0:
                        ps_o[key] = ps.tile(
                            [128, QG // 2], F32, tag="o", bufs=4,
                            name=f"ps_oh{half}"
                        )
                    for jj in range(4):
                        j = 4 * kk + jj
                        emit_pv_half(g, cb, j, csl,
                                     start=(j == 0), stop=(j == NM - 1), key=key)
                    if kk == 7:
                        emit_out_half(g, cb, csl, key)
                    continue
                if k == 0:
                    ps_o[(g, cb)] = ps.tile(
                        [128, QG], F32, tag="o", bufs=4, name=f"ps_o{g}_{cb}"
                    )
                emit_pv(g, cb, 2 * k, start=(k == 0), stop=False)
                emit_pv(g, cb, 2 * k + 1, start=False, stop=(k == 15))
                if k == 15:
                    emit_out(g, cb)

    # ---- startup (stream 0): S/exp/Z for group 0, cb0/cb1 transposes,
    # and the head of group 0's PV chains
    for t in range(NM):
        if t % 2 == 0:
            emit_s(0, t)
            emit_s(0, t + 1)
        if t % 4 == 0 and t >= 4:
            emit_z_quad(0, t // 4 - 1)
        if 10 <= t < 18:
            emit_transpose_quad(0, t - 10)
        if t >= 24:
            emit_transpose_quad(1, t - 24)
        chain_emits(0, t)

    # ---- phases p = 0..7 (stream s = p+1 slots)
    for T in range(8 * 32):
        p, t = divmod(T, 32)
        if t == 0:
            emit_z_quad(p, NM // 4 - 1)
            emit_rz(p)
        if p == 0 and 2 <= t < 10:
            emit_transpose_quad(2, t - 2)
        if p == 0 and 10 <= t < 18:
            emit_transpose_quad(3, t - 10)
        chain_emits(p + 1, t)
        if p + 1 <= 7 and t < NM:
            if t % 2 == 0:
                emit_s(p + 1, t)
                emit_s(p + 1, t + 1)
            if t % 4 == 0 and t >= 4:
                emit_z_quad(p + 1, t // 4 - 1)


_prog_cache = {}


def _get_program(repeat: int = 1):
    if repeat not in _prog_cache:
        _prog_cache[repeat] = build_program(repeat)
    return _prog_cache[repeat]


def run(inputs, **spmd_kwargs):
    from concourse.bass_utils import run_bass_kernel_spmd

    Mk = np.ascontiguousarray(np.asarray(inputs["Mk"], dtype=np.float32))
    Qk = np.ascontiguousarray(np.asarray(inputs["Qk"], dtype=np.float32))
    mv = np.ascontiguousarray(np.asarray(inputs["mv"], dtype=np.float32))
    assert Mk.shape == (B, CK, H, W) and Qk.shape == (B, CK, H, W)
    assert mv.shape == (B, CV, H, W)

    in_maps = [
        {
            "Mk": Mk[b].reshape(CK, HW),
            "Qk": Qk[b].reshape(CK, HW),
            "mv": mv[b].reshape(CV, HW),
        }
        for b in range(B)
    ]
    nc = _get_program()
    res = run_bass_kernel_spmd(nc, in_maps, list(range(B)), **spmd_kwargs)
    out = np.stack([res.results[b]["out"] for b in range(B)])
    return out.reshape(B, CV, H, W).astype(np.float32), res


def kernel(**inputs) -> np.ndarray:
    out, _ = run(inputs)
    return out


# revision 13
# speedup vs baseline: 254.5993x; 1.0006x over previous
"""Bass/Trainium2 kernel for nn_AttentionMemory (scatter_memory), v2.

Reference computation (per batch b):
    S   = Mk^T @ Qk * (1/sqrt(CK))     # [HW, HW]
    P   = softmax(S, axis=memory)      # softmax over the m (row) axis
    out = mv @ P                       # [CV, HW]

Sharding: B=8 batches, one batch per NeuronCore (pure data parallel).

v2 schedule: fine-grained slot interleave. The S/exp/Z stream for group
g+1 is woven between the PV accumulation matmuls of group g so the
Activation engine (exp, 612 ns/tile) runs concurrently with PE instead of
gating a separate S phase. PV chains are staggered across "flat slots"
(chain (g,cb) occupies flat slots 32g+8cb .. +15 at 2 matmuls/slot) so
PSUM drains + out-multiplies spread out instead of bunching at group
boundaries. Z colsums ride in distinct PE column groups (tile_position)
and S matmuls in the two K=64 row halves, which run concurrently on HW.
rz broadcast moved to the idle GPSIMD engine (partition_broadcast).
"""

import numpy as np

import concourse.bass as bass
import concourse.mybir as mybir
import concourse.tile as tile
from concourse.masks import make_identity
from bass_rust import ScopedClock

B, CK, CV, H, W = 8, 64, 512, 64, 64
HW = H * W            # 4096
QG = 512              # q-group width (one PSUM bank of fp32)
NQ = HW // QG         # 8 q-groups
NM = HW // 128        # 32 m-chunks
NCB = CV // 128       # 4 c-blocks
SCALE = 1.0 / 8.0     # 1/sqrt(CK)

F32 = mybir.dt.float32
FP16 = mybir.dt.float16
BF16 = mybir.dt.bfloat16


class FixedTileContext(tile.TileContext):
    """Splits multi-wait sync_infos: this walrus accepts at most one sync
    wait per regular instruction (two on InstEventSemaphore). Extra waits
    move onto same-engine InstNoOp carriers inserted just before."""

    def _split_multi_waits(self, ordered):
        nc = self.nc
        for bb_name, insts in list(ordered.items()):
            new_insts = []
            changed = False
            for inst in insts:
                si = getattr(inst, "sync_info", None)
                waits = list(si.on_wait) if (si is not None and si.on_wait) else []
                limit = 2 if isinstance(inst, mybir.InstEventSemaphore) else 1
                if len(waits) > limit:
                    changed = True
                    for w in waits[limit:]:
                        new_insts.append(
                            mybir.InstNoOp(
                                name=nc.get_next_instruction_name(),
                                sync_info=mybir.SyncInfo(on_wait=[w], on_update=[]),
                                bass_nofuse=True,
                                engine=inst.engine,
                            )
                        )
                    inst.sync_info = mybir.SyncInfo(
                        on_wait=waits[:limit], on_update=list(si.on_update or [])
                    )
                new_insts.append(inst)
            if changed:
                ordered[bb_name] = new_insts

    def _lower_ordered_insts(self, ordered):
        self._split_multi_waits(ordered)
        return super()._lower_ordered_insts(ordered)

    def _drain_and_barrier(self, tick_clock, wait_clock):
        nc = self.nc
        drain_inst = nc.sync.drain()
        wait_clock.add_sem_waits(
            drain_inst.ins, ScopedClock({None: tick_clock.global_clock})
        )
        si = drain_inst.ins.sync_info
        waits = list(si.on_wait or []) if si is not None else []
        if len(waits) > 1:
            drain_inst.ins.sync_info = mybir.SyncInfo(
                on_wait=[waits[0]], on_update=list(si.on_update or [])
            )
            for w in waits[1:]:
                d2 = nc.sync.drain()
                d2.ins.sync_info = mybir.SyncInfo(on_wait=[w], on_update=[])
        nc.all_engine_barrier()
        assert self.sems is not None
        popped = nc._tile_sem_poison_stack.pop()
        assert popped is self._sem_poison
        nc.clear_and_free_semaphores(list(self.sems.allocated().values()))
        nc.all_engine_barrier()


def build_program(repeat: int = 1) -> bass.Bass:
    nc = bass.Bass()
    mk_d = nc.dram_tensor("Mk", [CK, HW], F32, kind="ExternalInput")
    qk_d = nc.dram_tensor("Qk", [CK, HW], F32, kind="ExternalInput")
    mv_d = nc.dram_tensor("mv", [CV, HW], F32, kind="ExternalInput")
    out_d = nc.dram_tensor("out", [CV, HW], F32, kind="ExternalOutput")

    with FixedTileContext(nc) as tc:
        with (
            tc.tile_pool(name="consts", bufs=1) as consts,
            tc.tile_pool(name="stage", bufs=2) as stage,
            tc.tile_pool(name="inp16", bufs=1) as inp16,
            tc.tile_pool(name="mvtp", bufs=1) as mvtp,
            tc.tile_pool(name="pp", bufs=2) as pp,
            tc.tile_pool(name="obp", bufs=2) as obp,
            tc.tile_pool(name="smallp", bufs=2) as smallp,
            tc.tile_pool(name="ps", bufs=2, space="PSUM") as ps,
        ):
            identity = consts.tile([128, 128], F32)
            make_identity(nc, identity[:])
            ident16 = consts.tile([128, 128], BF16)
            nc.vector.tensor_copy(ident16[:], identity[:])

            ones_h = consts.tile([128, 1], BF16)
            nc.vector.memset(ones_h[:], 1.0)
            ones_r = consts.tile([1, 128], BF16)
            nc.gpsimd.memset(ones_r[:], 1.0)

            for _rep in range(repeat):
                emit_body(nc, tc, stage, inp16, mvtp, pp, obp, smallp, ps,
                          ident16, ones_h, ones_r, mk_d, qk_d, mv_d, out_d)
    return nc


def emit_body(nc, tc, stage, inp16, mvtp, pp, obp, smallp, ps,
              ident16, ones_h, ones_r, mk_d, qk_d, mv_d, out_d):
    # ---- HW warmup during the initial DMA wait (both invisible to the
    # cost-model sim, real on hardware):
    #  - dummy exp: pulls the ~1.3us activation-table load off the first
    #    real exp, which otherwise gates the S stream
    #  - dummy matmuls: keep the PE busy through the HAM activity window so
    #    the real S matmuls start at 2.4 GHz instead of the cold 1.2 GHz
    #    (PE-transposes don't count as HAM activity; matmuls do)
    warm_o = smallp.tile([128, 1], F32, tag="warm", bufs=1, name="warm_o")
    nc.scalar.activation(warm_o[:], ones_h[:],
                         mybir.ActivationFunctionType.Exp, scale=1.0)
    ps_warm = ps.tile([128, QG], F32, tag="s", name="ps_warm")
    for _ in range(16):
        nc.tensor.matmul(ps_warm[0:1, 0:1], ones_h[:], ones_h[:],
                         start=True, stop=True)
    for _ in range(60):
        nc.tensor.matmul(ps_warm[0:1, :128], ones_h[:], ident16[:],
                         start=True, stop=True)

    # ---- input load + cast to fp16, duplicated into both partition halves
    # (ch0 via double-DMA for latency; ch1-3 single-DMA + dup casts to save
    # DMA bandwidth for the mv loads). DMA order interleaves mv so every
    # consumer's data arrives just in time under aggregate-bandwidth limits.
    mk16 = inp16.tile([128, HW], FP16)
    qk16 = inp16.tile([128, HW], FP16)
    NCH = 4
    CW = HW // NCH
    mv_sb = []

    def emit_mv_dma(cb):
        t = stage.tile([128, HW], F32, tag="mv", name=f"mv_sb{cb}")
        nc.sync.dma_start(t[:], mv_d[cb * 128:(cb + 1) * 128, :])
        mv_sb.append(t)

    for ch in range(NCH):
        csl = slice(ch * CW, (ch + 1) * CW)
        for src_d, dst in ((mk_d, mk16), (qk_d, qk16)):
            if ch == 0:
                st = stage.tile([128, CW], F32, tag="mkqk")
                nc.sync.dma_start(st[:CK, :], src_d[:, csl])
                nc.sync.dma_start(st[CK:, :], src_d[:, csl])
                if dst is qk16:
                    # scalar engine is idle until the first exp (which
                    # transitively waits on this cast): run the two ch0
                    # casts in parallel on Act + DVE
                    nc.scalar.copy(dst[:, csl], st[:])
                else:
                    nc.vector.tensor_copy(dst[:, csl], st[:])
            else:
                st = stage.tile([64, CW], F32, tag="mkqk1")
                nc.sync.dma_start(st[:], src_d[:, csl])
                nc.vector.tensor_copy(dst[:CK, csl], st[:])
                nc.gpsimd.tensor_copy(dst[CK:, csl], st[:])
        if ch == 0:
            # first half of mv0 right after ch0: cb0's first 4 transpose
            # quads (m-chunks 0..15) can start ~4us earlier
            t0 = stage.tile([128, HW], F32, tag="mv", name="mv_sb0")
            nc.sync.dma_start(t0[:, :HW // 2], mv_d[0:128, :HW // 2])
            mv_sb.append(t0)
        elif ch == 2:
            nc.sync.dma_start(mv_sb[0][:, HW // 2:], mv_d[0:128, HW // 2:])
        elif ch == 3:
            for cb in range(1, NCB):
                emit_mv_dma(cb)

    # mvT[p, j, c] = mv[c, j*128+p], bf16 (PV stationary operand)
    mvT = mvtp.tile([128, NM, CV], BF16)

    P = [None] * NQ     # P[g]: [128, NM, QG] bf16, unnormalized exp
    ps_z = [None] * NQ  # Z colsum accumulators (4 col-group partials)
    rzb = [None] * NQ   # broadcast 1/Z rows
    ps_o = {}           # (g, cb) -> PV accumulation PSUM tile

    def emit_transpose_quad(cb, q):
        """Transpose m-chunks j=4q..4q+3 of mv c-block cb: 4 PE transposes
        into one PSUM tile, one DVE copy out (keeps the shared s-ring at
        ~2 allocs/slot)."""
        mq = stage.tile([128, QG], BF16, tag="mq", bufs=4, name="mq")
        nc.vector.tensor_copy(mq[:], mv_sb[cb][:, QG * q:QG * (q + 1)])
        ps_t = ps.tile([128, QG], BF16, tag="o", bufs=4, name="ps_t")
        for jj in range(4):
            nc.tensor.transpose(
                ps_t[:, jj * 128:(jj + 1) * 128],
                mq[:, jj * 128:(jj + 1) * 128], ident16[:]
            )
        nc.vector.tensor_copy(
            mvT[:, 4 * q:4 * q + 4, cb * 128:(cb + 1) * 128],
            ps_t.rearrange("p (j c) -> p j c", j=4),
        )

    def emit_s(g, j):
        """One S matmul + exp for (g, j). Allocates P[g]/ps_z[g] on j==0."""
        if j == 0:
            P[g] = pp.tile([128, NM, QG], BF16, tag="P", name=f"P{g}")
            ps_z[g] = ps.tile([128, QG], F32, tag="z", name=f"ps_z{g}")
        qsl = slice(g * QG, (g + 1) * QG)
        half = j % 2
        ksl = slice(half * CK, half * CK + CK)
        ps_sj = ps.tile([128, QG], F32, tag="s", name="ps_s")
        nc.tensor.matmul(
            ps_sj[:], mk16[ksl, j * 128:(j + 1) * 128], qk16[ksl, qsl],
            start=True, stop=True,
        )
        nc.scalar.activation(
            P[g][:, j, :], ps_sj[:],
            mybir.ActivationFunctionType.Exp, scale=SCALE,
        )

    def emit_z_quad(g, a):
        """Z colsum chunks j=4a..4a+3 for group g, emitted back-to-back so
        the 4 column-group chains run concurrently on the PE array."""
        for c in range(4):
            nc.tensor.matmul(
                ps_z[g][32 * c:32 * c + 1, :], ones_h[:], P[g][:, 4 * a + c, :],
                start=(a == 0), stop=(a == NM // 4 - 1),
                tile_position=(0, 32 * c),
            )

    def emit_rz(g):
        """Combine Z partials -> reciprocal -> broadcast (DVE + GPSIMD)."""
        za = smallp.tile([1, QG], F32, tag="zt", name="za")
        nc.vector.tensor_copy(za[:], ps_z[g][0:1, :])
        zb = smallp.tile([1, QG], F32, tag="zt", name="zb")
        nc.vector.tensor_tensor(
            out=zb[:], in0=za[:], in1=ps_z[g][32:33, :], op=mybir.AluOpType.add
        )
        zc = smallp.tile([1, QG], F32, tag="zt", name="zc")
        nc.vector.tensor_tensor(
            out=zc[:], in0=zb[:], in1=ps_z[g][64:65, :], op=mybir.AluOpType.add
        )
        zs = smallp.tile([1, QG], F32, tag="zt", name="zs")
        nc.vector.tensor_tensor(
            out=zs[:], in0=zc[:], in1=ps_z[g][96:97, :], op=mybir.AluOpType.add
        )
        rz = smallp.tile([1, QG], F32, tag="rz", name="rz")
        nc.vector.reciprocal(rz[:], zs[:])
        rz16 = smallp.tile([1, QG], BF16, tag="rz16", name="rz16")
        nc.vector.tensor_copy(rz16[:], rz[:])
        # broadcast along partitions: ones[1,128]^T @ rz16[1,QG] (bf16, 213ns)
        ps_rzb = ps.tile([128, QG], F32, tag="s", name="ps_rzb")
        nc.tensor.matmul(ps_rzb[:], ones_r[:], rz16[:], start=True, stop=True)
        rzb[g] = smallp.tile([128, QG], F32, tag="rzb", name=f"rzb{g}")
        nc.vector.tensor_copy(rzb[g][:], ps_rzb[:])

    def emit_pv(g, cb, j, start, stop):
        nc.tensor.matmul(
            ps_o[(g, cb)][:],
            mvT[:, j, cb * 128:(cb + 1) * 128],
            P[g][:, j, :],
            start=start, stop=stop,
        )

    def emit_out(g, cb):
        qsl = slice(g * QG, (g + 1) * QG)
        o_sb = obp.tile([128, QG], F32, tag="ob", name="o_sb")
        nc.vector.tensor_tensor(
            out=o_sb[:], in0=ps_o.pop((g, cb))[:], in1=rzb[g][:],
            op=mybir.AluOpType.mult,
        )
        nc.sync.dma_start(out_d[cb * 128:(cb + 1) * 128, qsl], o_sb[:])

    def emit_pv_half(g, cb, j, csl, start, stop, key):
        nc.tensor.matmul(
            ps_o[key][:],
            mvT[:, j, cb * 128:(cb + 1) * 128],
            P[g][:, j, csl],
            start=start, stop=stop,
        )

    def emit_out_half(g, cb, csl, key):
        qs = slice(g * QG + csl.start, g * QG + csl.stop)
        o_sb = obp.tile([128, csl.stop - csl.start], F32, tag="obh", bufs=2,
                        name="o_sbh")
        nc.vector.tensor_tensor(
            out=o_sb[:], in0=ps_o.pop(key)[:], in1=rzb[g][:, csl],
            op=mybir.AluOpType.mult,
        )
        nc.sync.dma_start(out_d[cb * 128:(cb + 1) * 128, qs], o_sb[:])

    def chain_emits(s, t):
        """PV chain work due at slot t of stream s. Chain (g, cb) occupies
        stream-g slots 18+8cb .. 31 and stream-(g+1) slots 0 .. 8cb+1.
        The very last chain (NQ-1, cb3) runs as two sequential q-halves so
        half A's out-mult + DMA overlap half B's matmuls (shorter tail)."""
        for cb in range(NCB):
            for g, k in ((s, t - 18 - 8 * cb), (s - 1, 32 + t - 18 - 8 * cb)):
                if not (0 <= g < NQ and 0 <= k < 16):
                    continue
                if g == NQ - 1 and cb == 3:
                    # last chain runs as four sequential q-quarters: each
                    # quarter's out-mult + DMA overlap the next quarter's
                    # matmuls, so only the final [128,128] drain is exposed
                    part = k // 4
                    csl = slice(part * (QG // 4), (part + 1) * (QG // 4))
                    key = (g, cb, part)
                    kk = k % 4
                    if kk == 0:
                        ps_o[key] = ps.tile(
                            [128, QG // 4], F32, tag="o", bufs=4,
                            name=f"ps_oq{part}"
                        )
                    for jj in range(8):
                        j = 8 * kk + jj
                        emit_pv_half(g, cb, j, csl,
                                     start=(j == 0), stop=(j == NM - 1), key=key)
                    if kk == 3:
                        emit_out_half(g, cb, csl, key)
                    continue
                if k == 0:
                    ps_o[(g, cb)] = ps.tile(
                        [128, QG], F32, tag="o", bufs=4, name=f"ps_o{g}_{cb}"
                    )
                emit_pv(g, cb, 2 * k, start=(k == 0), stop=False)
                emit_pv(g, cb, 2 * k + 1, start=False, stop=(k == 15))
                if k == 15:
                    emit_out(g, cb)

    # ---- startup (stream 0): S/exp/Z for group 0, cb0/cb1 transposes,
    # and the head of group 0's PV chains
    for t in range(NM):
        if t % 2 == 0:
            emit_s(0, t)
            emit_s(0, t + 1)
        if t % 4 == 0 and t >= 4:
            emit_z_quad(0, t // 4 - 1)
        if 10 <= t < 18:
            emit_transpose_quad(0, t - 10)
        if t >= 24:
            emit_transpose_quad(1, t - 24)
        chain_emits(0, t)

    # ---- phases p = 0..7 (stream s = p+1 slots)
    for T in range(8 * 32):
        p, t = divmod(T, 32)
        if t == 0:
            emit_z_quad(p, NM // 4 - 1)
            emit_rz(p)
        if p == 0 and 2 <= t < 10:
            emit_transpose_quad(2, t - 2)
        if p == 0 and 10 <= t < 18:
            emit_transpose_quad(3, t - 10)
        chain_emits(p + 1, t)
        if p + 1 <= 7 and t < NM:
            if t % 2 == 0:
                emit_s(p + 1, t)
                emit_s(p + 1, t + 1)
            if t % 4 == 0 and t >= 4:
                emit_z_quad(p + 1, t // 4 - 1)


_prog_cache = {}


def _get_program(repeat: int = 1):
    if repeat not in _prog_cache:
        _prog_cache[repeat] = build_program(repeat)
    return _prog_cache[repeat]


def run(inputs, **spmd_kwargs):
    from concourse.bass_utils import run_bass_kernel_spmd

    Mk = np.ascontiguousarray(np.asarray(inputs["Mk"], dtype=np.float32))
    Qk = np.ascontiguousarray(np.asarray(inputs["Qk"], dtype=np.float32))
    mv = np.ascontiguousarray(np.asarray(inputs["mv"], dtype=np.float32))
    assert Mk.shape == (B, CK, H, W) and Qk.shape == (B, CK, H, W)
    assert mv.shape == (B, CV, H, W)

    in_maps = [
        {
            "Mk": Mk[b].reshape(CK, HW),
            "Qk": Qk[b].reshape(CK, HW),
            "mv": mv[b].reshape(CV, HW),
        }
        for b in range(B)
    ]
    nc = _get_program()
    res = run_bass_kernel_spmd(nc, in_maps, list(range(B)), **spmd_kwargs)
    out = np.stack([res.results[b]["out"] for b in range(B)])
    return out.reshape(B, CV, H, W).astype(np.float32), res


def kernel(**inputs) -> np.ndarray:
    out, _ = run(inputs)
    return out
